# revision 1
# baseline (speedup 1.0000x reference)
"""Dense transformer block (nn_Block_87127706566879) on 8 TRN2 NeuronCores.

Sharding: DP over batch (4 pairs) x Megatron TP=2 within each core pair.
Each core handles one batch element, 8 of 16 heads, and half of the MLP
FF dim. One AllReduce (per core pair) after the attention out-proj and
one after fc2, both split into two sequence-halves so collectives
overlap compute.

The residual stream lives TRANSPOSED on chip ([E, S], e on partitions).
LayerNorm stats (over e = partition dim) are computed with ones-vector
matmuls on the PE; per-s stats are broadcast back across partitions with
a K=1 ones matmul. LN gains/biases are folded into the following matmul
weights host-side; residual scale factors (x/4, w_o/2, b_fc2/2, ...) are
folded so the two AllReduces reconstruct exact sums with no extra
elementwise passes.

Matmuls run in float32r (fp32 storage, ~tf32 precision, full PE rate at
free-dim >= 256). Attention: scores are computed transposed
(S^T = [k, q], keys stationary, two heads row-packed via tile_position);
softmax exp runs on ACT with the periodic (k % td == td-1) mask folded
into the per-partition bias and the 1/sqrt(dh) scale folded into the
activation scale; causal masking of diagonal tiles is a 0/1 mask
multiply on the DVE. The AV matmul appends a ones column to V
([V | 1], M=65) so the softmax denominator accumulates for free in PSUM
row 64; normalization happens once on the (much smaller) attn output.
"""

import math
from contextlib import ExitStack
from dataclasses import dataclass

import numpy as np

import concourse.bass as bass
import concourse.tile as tile
from concourse import bacc, mybir
from concourse._compat import with_exitstack

F32 = mybir.dt.float32
F32R = mybir.dt.float32r
AF = mybir.ActivationFunctionType
OP = mybir.AluOpType
NEG = -1e30


@dataclass(frozen=True)
class Cfg:
    B: int = 4
    S: int = 2048
    E: int = 1024
    H: int = 16
    FF: int = 4096
    n_pairs: int = 4  # cores = 2 * n_pairs
    gelu_exact: bool = True  # False: x*sigmoid(1.702x) (CoreSim lacks Gelu)

    @property
    def Dh(self):
        return self.E // self.H

    @property
    def HL(self):
        return self.H // 2  # heads per core

    @property
    def HP(self):
        return self.HL // 2  # head pairs per core

    @property
    def OL(self):
        return self.HL * self.Dh  # attn out dims per core

    @property
    def FL(self):
        return self.FF // 2  # ff dims per core

    @property
    def ET(self):
        return self.E // 128

    @property
    def ST(self):
        return self.S // 128

    @property
    def SH(self):
        return self.S // 2  # sequence half

    @property
    def SC(self):
        return self.SH // 512  # 512-chunks per half

    @property
    def C4(self):
        return self.S // 512  # q-chunks

    @property
    def FT(self):
        return self.FL // 128

    @property
    def OT(self):
        return self.OL // 128  # attn o-tiles (= head pairs)


@with_exitstack
def block_kernel(ctx: ExitStack, tc: tile.TileContext, cfg: Cfg, ins, outs):
    nc = tc.nc
    ET, SH, SC, C4, FT, HP, OT = (
        cfg.ET, cfg.SH, cfg.SC, cfg.C4, cfg.FT, cfg.HP, cfg.OT
    )
    S, E, OL = cfg.S, cfg.E, cfg.OL
    ST = cfg.ST
    eps1 = 1e-5 / 16.0  # LN1 runs on x/4
    eps2 = 1e-5 / 4.0   # LN2 runs on x2/2
    groups = [[2 * p, 2 * p + 1] for p in range(cfg.n_pairs)]

    cst = ctx.enter_context(tc.tile_pool(name="cst", bufs=1))

    def load_const(name, dt_):
        t = cst.tile(list(ins[name].shape), dt_, tag=name)
        nc.sync.dma_start(t[:], ins[name])
        return t

    onesrow = load_const("onesrow", F32R)      # [1, 512]
    onehot2 = load_const("onehot2", F32R)      # [2, 128]
    ones128 = load_const("ones128", F32R)      # [128, 8] (col 0: stats lhsT)
    maskd = load_const("maskd", F32R)          # [128, 128] 0/1 lower-tri (k<=q)
    pbias = load_const("pbias", F32)           # [128, ST] periodic -1e30 bias
    bqk = load_const("bqk", F32)               # [128, 2*OT]
    bv = load_const("bv", F32R)                # [1, OL]
    bo_q = load_const("bo_q", F32R)            # [1, E]
    bfc1 = load_const("bfc1", F32)             # [128, FT]
    bfc2_q = load_const("bfc2_q", F32R)        # [1, E]


    def ln_stats_apply(pools, x_tiles, sc_off, eps, out_tiles, out_off,
                       apply_tiles=None, ap_off=None):
        """LN over the partition (e) dim for one 512-col chunk of x^T.

        Stats come from x_tiles (f32r-viewable); the normalization is
        applied to apply_tiles (defaults to x_tiles; may alias out_tiles
        for in-place).
        """
        ps_stat, ps_bb, sb_small, sb_big, sb_bc = pools
        if apply_tiles is None:
            apply_tiles, ap_off = x_tiles, sc_off
        n = 512
        psum_s = ps_stat.tile([1, n], F32, tag="sum", name="sum")
        psum_q = ps_stat.tile([1, n], F32, tag="sq", name="sq")
        for et in range(ET):
            xr = x_tiles[et][:, sc_off : sc_off + n]
            sq = sb_big.tile([128, n], F32R, tag="sqt", name="sqt")
            nc.vector.tensor_mul(sq[:], xr.bitcast(F32), xr.bitcast(F32))
            nc.tensor.matmul(
                psum_s[:], lhsT=ones128[:, 0:1], rhs=xr,
                start=(et == 0), stop=(et == ET - 1),
            )
            nc.tensor.matmul(
                psum_q[:], lhsT=ones128[:, 0:1], rhs=sq[:],
                start=(et == 0), stop=(et == ET - 1),
            )
        inv_e = 1.0 / cfg.E
        m = sb_small.tile([1, n], F32R, tag="m", name="m")
        nc.vector.tensor_scalar(m[:], psum_s[:], inv_e, None, OP.mult)
        var = sb_small.tile([1, n], F32, tag="var", name="var")
        nc.vector.tensor_scalar(var[:], psum_q[:], inv_e, None, OP.mult)
        t1 = sb_small.tile([1, n], F32, tag="t1", name="t1")
        nc.vector.tensor_mul(t1[:], m[:].bitcast(F32), m[:].bitcast(F32))
        nc.vector.tensor_sub(var[:], var[:], t1[:])
        nc.vector.tensor_scalar(var[:], var[:], eps, None, OP.add)
        # rstd = rsqrt(var): ACT sqrt + DVE recip + one Newton step
        # (ACT sqrt has a loose ULP budget).
        nc.scalar.sqrt(t1[:], var[:])
        y = sb_small.tile([1, n], F32, tag="y", name="y")
        nc.vector.reciprocal(y[:], t1[:])
        nc.vector.tensor_mul(t1[:], y[:], y[:])
        nc.vector.tensor_mul(t1[:], t1[:], var[:])
        nc.vector.tensor_scalar(t1[:], t1[:], -0.5, 1.5, OP.mult, OP.add)
        rstd = sb_small.tile([1, n], F32R, tag="rstd", name="rstd")
        nc.vector.tensor_mul(rstd[:], y[:], t1[:])
        # broadcast m, rstd across partitions via K=1 ones matmul
        pm = ps_bb.tile([128, n], F32, tag="pm", name="pm")
        nc.tensor.matmul(pm[:], lhsT=onesrow[:, 0:128], rhs=m[:], start=True, stop=True)
        pr = ps_bb.tile([128, n], F32, tag="pr", name="pr")
        nc.tensor.matmul(pr[:], lhsT=onesrow[:, 0:128], rhs=rstd[:], start=True, stop=True)
        m_b = sb_bc.tile([128, n], F32, tag="m_b", name="m_b")
        nc.vector.tensor_copy(m_b[:], pm[:])
        r_b = sb_bc.tile([128, n], F32, tag="r_b", name="r_b")
        nc.vector.tensor_copy(r_b[:], pr[:])
        for et in range(ET):
            src_ap = apply_tiles[et][:, ap_off : ap_off + n]
            out_ap = out_tiles[et][:, out_off : out_off + n]
            nc.vector.tensor_sub(out_ap, src_ap.bitcast(F32), m_b[:])
            nc.vector.tensor_mul(out_ap, out_ap, r_b[:])

    # ---------------- LN1 + QKV (qkT/VO persist into attention) ----------------
    qk_vo_stack = ExitStack()
    qk_pool = qk_vo_stack.enter_context(tc.tile_pool(name="qk", bufs=1))
    qkT = [qk_pool.tile([128, S], F32R, tag=f"qkT{i}", name=f"qkT{i}") for i in range(2 * OT)]
    vo_pool = qk_vo_stack.enter_context(tc.tile_pool(name="vo", bufs=1))
    VO = [vo_pool.tile([128, cfg.HL * 65], F32R, tag=f"vo{i}", name=f"vo{i}") for i in range(ST)]
    with (
        tc.tile_pool(name="xq_sb", bufs=1) as xq_pool,
        tc.tile_pool(name="wqk", bufs=1) as wqk_pool,
        tc.tile_pool(name="wv", bufs=1) as wv_pool,
        tc.tile_pool(name="ps_qk", bufs=2, space="PSUM") as ps_qk,
        tc.tile_pool(name="ps_v", bufs=2, space="PSUM") as ps_v,
        tc.tile_pool(name="ps_stat", bufs=1, space="PSUM") as ps_stat,
        tc.tile_pool(name="ps_bb", bufs=1, space="PSUM") as ps_bb,
        tc.tile_pool(name="sb_small", bufs=1) as sb_small,
        tc.tile_pool(name="sb_big", bufs=2) as sb_big,
        tc.tile_pool(name="sb_bc", bufs=1) as sb_bc,
    ):
        ln_pools = (ps_stat, ps_bb, sb_small, sb_big, sb_bc)
        wqk_sb = wqk_pool.tile([128, ET, 2 * OL], F32R, tag="wqk", name="wqk")
        nc.sync.dma_start(
            wqk_sb[:], ins["wqkT"].rearrange("(et p) o -> p et o", p=128)
        )
        wv_sb = wv_pool.tile([128, ET, OL], F32R, tag="wv", name="wv")
        nc.sync.dma_start(
            wv_sb[:], ins["wvT"].rearrange("(et p) o -> p et o", p=128)
        )
        for half in range(2):
            hoff = half * SH
            for sc in range(SC):
                coff = sc * 512
                xq_sb = [
                    xq_pool.tile([128, 512], F32R, tag=f"xq{et}", name=f"xq{et}")
                    for et in range(ET)
                ]
                for et in range(ET):
                    nc.sync.dma_start(
                        xq_sb[et][:],
                        ins["xq"][
                            et * 128 : (et + 1) * 128,
                            hoff + coff : hoff + coff + 512,
                        ],
                    )
                xn1 = xq_sb  # LN1 applied in place
                ln_stats_apply(ln_pools, xq_sb, 0, eps1, xn1, 0)
                # Q,K projections: out qkT [o, s] (w stationary)
                for ot in range(2 * OT):
                    psum = ps_qk.tile([128, 512], F32, tag="qk", name="qk")
                    for et in range(ET):
                        nc.tensor.matmul(
                            psum[:],
                            lhsT=wqk_sb[:, et, ot * 128 : (ot + 1) * 128],
                            rhs=xn1[et][:],
                            start=(et == 0),
                            stop=(et == ET - 1),
                        )
                    nc.vector.tensor_scalar(
                        qkT[ot][:, hoff + coff : hoff + coff + 512],
                        psum[:],
                        bqk[:, ot : ot + 1],
                        None,
                        OP.add,
                    )
                # V projection: out V [s, o_v] (xn1 stationary), bias preloaded
                for stl in range(4):  # s-tiles within this 512-chunk
                    st = (hoff + coff) // 128 + stl
                    psum = ps_v.tile([128, OL], F32, tag="v", name="v")
                    # bias broadcast preload: out[sp, o] = 1 * bv[o]
                    nc.tensor.matmul(
                        psum[:, 0:OL], lhsT=onesrow[:, 0:128], rhs=bv[:],
                        start=True, stop=False,
                    )
                    for et in range(ET):
                        nc.tensor.matmul(
                            psum[:],
                            lhsT=xn1[et][:, stl * 128 : (stl + 1) * 128],
                            rhs=wv_sb[:, et],
                            start=False,
                            stop=(et == ET - 1),
                        )
                    for h in range(cfg.HL):
                        nc.vector.tensor_copy(
                            VO[st][:, h * 65 : h * 65 + 64],
                            psum[:, h * 64 : (h + 1) * 64],
                        )
                    nc.vector.tensor_copy(
                        VO[st][:, 64 :: 65], ones128[:, 0 : cfg.HL]
                    )

    # ---------------- attention ----------------
    at_stack = ExitStack()
    at_pool = at_stack.enter_context(tc.tile_pool(name="attnT", bufs=1))
    attnT = [at_pool.tile([128, S], F32R, tag=f"at{i}", name=f"at{i}") for i in range(OT)]
    with (
        tc.tile_pool(name="pt", bufs=3) as pt_pool,
        tc.tile_pool(name="den", bufs=2) as den_pool,
        tc.tile_pool(name="ps_sc", bufs=2, space="PSUM") as ps_sc,
        tc.tile_pool(name="ps_av", bufs=1, space="PSUM") as ps_av,
        tc.tile_pool(name="ps_bc", bufs=1, space="PSUM") as ps_bc,
    ):
        scale = 1.0 / math.sqrt(cfg.Dh)
        for c in range(C4):
            kmax = 4 * c + 4
            for hp in range(HP):
                av = [ps_av.tile([65, 512], F32, tag=f"av{h01}", name=f"av{h01}") for h01 in range(2)]
                for i in range(kmax):
                    psc = ps_sc.tile([128, 1024], F32, tag="sc", name="sc")
                    for h01 in range(2):
                        po = h01 * 64
                        nc.tensor.matmul(
                            psc[:, h01 * 512 : (h01 + 1) * 512],
                            lhsT=qkT[OT + hp][po : po + 64, i * 128 : (i + 1) * 128],
                            rhs=qkT[hp][po : po + 64, c * 512 : (c + 1) * 512],
                            start=True,
                            stop=True,
                            tile_position=(po, 0),
                        )
                    pt = pt_pool.tile([128, 1024], F32R, tag="pt", name="pt")
                    nc.scalar.activation(
                        pt[:], psc[:], AF.Exp,
                        bias=pbias[:, i : i + 1], scale=scale,
                    )
                    r = i - 4 * c
                    if r >= 0:
                        for h01 in range(2):
                            if r > 0:
                                zsl = pt[:, h01 * 512 : h01 * 512 + r * 128]
                                nc.vector.tensor_scalar(
                                    zsl, zsl, 0.0, None, OP.mult
                                )
                            sl = slice(h01 * 512 + r * 128, h01 * 512 + (r + 1) * 128)
                            nc.vector.tensor_mul(pt[:, sl], pt[:, sl], maskd[:])
                    for h01 in range(2):
                        hloc = 2 * hp + h01
                        nc.tensor.matmul(
                            av[h01][:],
                            lhsT=VO[i][:, hloc * 65 : (hloc + 1) * 65],
                            rhs=pt[:, h01 * 512 : (h01 + 1) * 512],
                            start=(i == 0),
                            stop=(i == kmax - 1),
                        )
                dens = [
                    den_pool.tile([1, 512], F32R, tag=f"den{h01}", name=f"den{h01}")
                    for h01 in range(2)
                ]
                with nc.allow_low_precision(reason="f32r rounding for matmul rhs"):
                    for h01 in range(2):
                        nc.vector.reciprocal(dens[h01][:], av[h01][64:65, :])
                # assemble [2, 512] (DMA can place row 1; DVE cannot)
                den2 = den_pool.tile([2, 512], F32R, tag="den2", name="den2")
                for h01 in range(2):
                    nc.sync.dma_start(den2[h01 : h01 + 1, :], dens[h01][:])
                pbc = ps_bc.tile([128, 512], F32, tag="bc", name="bc")
                nc.tensor.matmul(
                    pbc[:], lhsT=onehot2[:], rhs=den2[:], start=True, stop=True
                )
                sbc = den_pool.tile([128, 512], F32, tag="sbc", name="sbc")
                nc.vector.tensor_copy(sbc[:], pbc[:])
                for h01 in range(2):
                    nc.vector.tensor_mul(
                        attnT[hp][h01 * 64 : (h01 + 1) * 64, c * 512 : (c + 1) * 512],
                        av[h01][0:64, :],
                        sbc[h01 * 64 : (h01 + 1) * 64, :],
                    )

    # ---------------- out-proj + AR1 ----------------
    ar1_in = nc.dram_tensor("ar1_in", [2, E, SH], F32)
    ar1_out = nc.dram_tensor("ar1_out", [2, E, SH], F32)
    with (
        tc.tile_pool(name="wo", bufs=1) as wo_pool,
        tc.tile_pool(name="xqs", bufs=4) as xqs_pool,
        tc.tile_pool(name="arin", bufs=4) as arin_pool,
        tc.tile_pool(name="ps_o", bufs=2, space="PSUM") as ps_o,
    ):
        bo_q = wo_pool.tile([1, E], F32R, tag="bo_q", name="bo_q")
        nc.sync.dma_start(bo_q[:], ins["bo_q"])
        wo_sb = wo_pool.tile([128, OT, E], F32R, tag="wo", name="wo")
        nc.sync.dma_start(wo_sb[:], ins["woT_q"].rearrange("(ot p) e -> p ot e", p=128))
        for half in range(2):
            hoff = half * SH
            for et in range(ET):
                for sc in range(SC):
                    coff = hoff + sc * 512
                    psum = ps_o.tile([128, 512], F32, tag="o", name="o")
                    nc.tensor.matmul(
                        psum[:],
                        lhsT=bo_q[:, et * 128 : (et + 1) * 128],
                        rhs=onesrow[:],
                        start=True,
                        stop=False,
                    )
                    for ot in range(OT):
                        nc.tensor.matmul(
                            psum[:],
                            lhsT=wo_sb[:, ot, et * 128 : (et + 1) * 128],
                            rhs=attnT[ot][:, coff : coff + 512],
                            start=False,
                            stop=(ot == OT - 1),
                        )
                    xqs = xqs_pool.tile([128, 512], F32, tag="xqs", name="xqs")
                    nc.sync.dma_start(
                        xqs[:],
                        ins["xq"].bitcast(F32)[
                            et * 128 : (et + 1) * 128, coff : coff + 512
                        ],
                    )
                    arin = arin_pool.tile([128, 512], F32, tag="arin", name="arin")
                    nc.vector.tensor_add(arin[:], psum[:], xqs[:])
                    nc.sync.dma_start(
                        ar1_in[
                            half, et * 128 : (et + 1) * 128, sc * 512 : sc * 512 + 512
                        ],
                        arin[:],
                    )
            nc.gpsimd.collective_compute(
                "AllReduce",
                OP.add,
                replica_groups=groups,
                ins=[ar1_in[half]],
                outs=[ar1_out[half]],
            )

    at_stack.close()   # attnT no longer needed
    qk_vo_stack.close()  # qkT/VO no longer needed

    # ---------------- LN2 + MLP + AR2 ----------------
    ar2_in = nc.dram_tensor("ar2_in", [2, E, SH], F32)
    ar2_out = nc.dram_tensor("ar2_out", [2, E, SH], F32)
    with (
        tc.tile_pool(name="x2h", bufs=1) as x2h_pool,
        tc.tile_pool(name="x2hr", bufs=2) as x2hr_pool,
        tc.tile_pool(name="x2res", bufs=3) as x2res_pool,
        tc.tile_pool(name="xn2", bufs=1) as xn2_pool,
        tc.tile_pool(name="ht", bufs=1) as ht_pool,
        tc.tile_pool(name="w1", bufs=3) as w1_pool,
        tc.tile_pool(name="w2", bufs=2) as w2_pool,
        tc.tile_pool(name="fin", bufs=2) as fin_pool,
        tc.tile_pool(name="ps_f1", bufs=2, space="PSUM") as ps_f1,
        tc.tile_pool(name="ps_f2", bufs=2, space="PSUM") as ps_f2,
        tc.tile_pool(name="ps_stat", bufs=1, space="PSUM") as ps_stat,
        tc.tile_pool(name="ps_bb", bufs=1, space="PSUM") as ps_bb,
        tc.tile_pool(name="sb_small", bufs=1) as sb_small,
        tc.tile_pool(name="sb_big", bufs=2) as sb_big,
        tc.tile_pool(name="sb_bc", bufs=1) as sb_bc,
    ):
        ln_pools = (ps_stat, ps_bb, sb_small, sb_big, sb_bc)
        bfc2_q = w2_pool.tile([1, E], F32R, tag="bfc2_q", name="bfc2_q")
        nc.sync.dma_start(bfc2_q[:], ins["bfc2_q"])
        for half in range(2):
            xn2 = [xn2_pool.tile([128, SH], F32R, tag=f"xn2{et}", name=f"xn2{et}") for et in range(ET)]
            for sc in range(SC):
                coff = sc * 512
                x2h = [
                    x2h_pool.tile([128, 512], F32, tag=f"x2h{et}", name=f"x2h{et}")
                    for et in range(ET)
                ]
                x2hr = [
                    x2hr_pool.tile([128, 512], F32R, tag=f"x2hr{et % 2}", name=f"x2hr{et % 2}")
                    for et in range(ET)
                ]
                for et in range(ET):
                    nc.sync.dma_start(
                        x2h[et][:],
                        ar1_out[
                            half, et * 128 : (et + 1) * 128, coff : coff + 512
                        ],
                    )
                    nc.vector.tensor_copy(x2hr[et][:], x2h[et][:])
                ln_stats_apply(ln_pools, x2hr, 0, eps2, xn2, coff,
                               apply_tiles=x2h, ap_off=0)
            ht_tiles = []
            for ft in range(FT):
                w1t = w1_pool.tile([128, ET, 128], F32R, tag="w1", name="w1")
                nc.sync.dma_start(
                    w1t[:],
                    ins["wfc1T"][:, ft * 128 : (ft + 1) * 128].rearrange(
                        "(et p) f -> p et f", p=128
                    ),
                )
                ht = ht_pool.tile([128, SH], F32R, tag=f"ht{ft}", name=f"ht{ft}")
                for sc in range(SC):
                    psum = ps_f1.tile([128, 512], F32, tag="f1", name="f1")
                    for et in range(ET):
                        nc.tensor.matmul(
                            psum[:],
                            lhsT=w1t[:, et],
                            rhs=xn2[et][:, sc * 512 : (sc + 1) * 512],
                            start=(et == 0),
                            stop=(et == ET - 1),
                        )
                    hsl = ht[:, sc * 512 : (sc + 1) * 512]
                    if cfg.gelu_exact:
                        nc.scalar.activation(
                            hsl, psum[:], AF.Gelu,
                            bias=bfc1[:, ft : ft + 1], scale=1.0,
                        )
                    else:
                        tg = fin_pool.tile([128, 512], F32, tag="tg", name="tg")
                        nc.vector.tensor_scalar(
                            tg[:], psum[:], bfc1[:, ft : ft + 1], None, OP.add
                        )
                        sg = fin_pool.tile([128, 512], F32, tag="sg", name="sg")
                        nc.scalar.activation(sg[:], tg[:], AF.Sigmoid, scale=1.702)
                        nc.vector.tensor_mul(hsl, tg[:], sg[:])
                ht_tiles.append(ht)
            for et in range(ET):
                w2t = w2_pool.tile([128, FT, 128], F32R, tag="w2", name="w2")
                nc.sync.dma_start(
                    w2t[:],
                    ins["wfc2T"][:, et * 128 : (et + 1) * 128].rearrange(
                        "(ft p) e -> p ft e", p=128
                    ),
                )
                fin = fin_pool.tile([128, SH], F32, tag="fin", name="fin")
                for sc in range(SC):
                    psum = ps_f2.tile([128, 512], F32, tag="f2", name="f2")
                    nc.tensor.matmul(
                        psum[:],
                        lhsT=bfc2_q[:, et * 128 : (et + 1) * 128],
                        rhs=onesrow[:],
                        start=True,
                        stop=False,
                    )
                    for ft in range(FT):
                        nc.tensor.matmul(
                            psum[:],
                            lhsT=w2t[:, ft],
                            rhs=ht_tiles[ft][:, sc * 512 : (sc + 1) * 512],
                            start=False,
                            stop=(ft == FT - 1),
                        )
                    x2res = x2res_pool.tile([128, 512], F32, tag="x2res", name="x2res")
                    nc.sync.dma_start(
                        x2res[:],
                        ar1_out[
                            half, et * 128 : (et + 1) * 128,
                            sc * 512 : (sc + 1) * 512,
                        ],
                    )
                    nc.vector.tensor_add(
                        fin[:, sc * 512 : (sc + 1) * 512], psum[:], x2res[:]
                    )
                nc.sync.dma_start(ar2_in[half, et * 128 : (et + 1) * 128, :], fin[:])
            nc.gpsimd.collective_compute(
                "AllReduce",
                OP.add,
                replica_groups=groups,
                ins=[ar2_in[half]],
                outs=[ar2_out[half]],
            )
            nc.sync.dma_start(outs["outT"][half], ar2_out[half])


# ---------------------------------------------------------------------------
# host side
# ---------------------------------------------------------------------------

def prep_inputs(cfg: Cfg, x, td, ln1_g, ln1_b, ln2_g, ln2_b, w_qkv, b_qkv,
                w_o, b_o, w_fc1, b_fc1, w_fc2, b_fc2):
    """Build the per-core input maps (numpy, fp32)."""
    E, H, FL, OL, HL = cfg.E, cfg.H, cfg.FL, cfg.OL, cfg.HL
    f4 = np.float32
    asc = np.ascontiguousarray

    wq, wk, wv = w_qkv[0:E], w_qkv[E : 2 * E], w_qkv[2 * E : 3 * E]
    bq, bk, bvv = b_qkv[0:E], b_qkv[E : 2 * E], b_qkv[2 * E : 3 * E]

    shared = {}
    shared["onesrow"] = np.ones((1, 512), f4)
    oh = np.zeros((2, 128), f4)
    oh[0, 0:64] = 1.0
    oh[1, 64:128] = 1.0
    shared["onehot2"] = oh
    shared["ones128"] = np.ones((128, 8), f4)
    k_idx = np.arange(128)
    shared["maskd"] = asc((k_idx[:, None] <= k_idx[None, :]).astype(f4))
    pb = np.zeros((128, cfg.ST), f4)
    for i in range(cfg.ST):
        kabs = i * 128 + k_idx
        pb[(kabs % td) == (td - 1), i] = NEG
    shared["pbias"] = pb
    shared["bo_q"] = asc(0.25 * b_o[None, :].astype(f4))
    shared["bfc2_q"] = asc(0.5 * b_fc2[None, :].astype(f4))

    per_tp = []
    for tp in range(2):
        o_sl = slice(tp * OL, (tp + 1) * OL)
        f_sl = slice(tp * FL, (tp + 1) * FL)
        d = {}
        wqk = np.concatenate([wq[o_sl], wk[o_sl]], axis=0)  # [2*OL, E]
        d["wqkT"] = asc((wqk * ln1_g[None, :]).T.astype(f4))  # [E, 2*OL]
        bqk_full = (
            np.concatenate([bq[o_sl], bk[o_sl]]) + wqk @ ln1_b
        ).astype(f4)  # [2*OL]
        d["bqk"] = asc(bqk_full.reshape(2 * cfg.OT, 128).T)  # [128, 2*OT]
        d["wvT"] = asc((wv[o_sl] * ln1_g[None, :]).T.astype(f4))  # [E, OL]
        d["bv"] = asc((bvv[o_sl] + wv[o_sl] @ ln1_b)[None, :].astype(f4))  # [1, OL]
        d["woT_q"] = asc(0.5 * w_o[:, o_sl].T.astype(f4))  # [OL, E]
        d["wfc1T"] = asc((w_fc1[f_sl] * ln2_g[None, :]).T.astype(f4))  # [E, FL]
        d["bfc1"] = asc(
            (b_fc1[f_sl] + w_fc1[f_sl] @ ln2_b).astype(f4).reshape(cfg.FT, 128).T
        )  # [128, FT]
        d["wfc2T"] = asc(w_fc2[:, f_sl].T.astype(f4))  # [FL, E]
        per_tp.append(d)

    in_maps = []
    for c in range(2 * cfg.n_pairs):
        p, tp = c // 2, c % 2
        m = dict(shared)
        m.update(per_tp[tp])
        m["xq"] = asc(0.25 * x[p].T.astype(f4))  # [E, S]
        in_maps.append(m)
    return in_maps


_F32R_INPUTS = {
    "xq", "wqkT", "wvT", "bv", "woT_q", "bo_q", "wfc1T", "wfc2T", "bfc2_q",
    "onesrow", "onehot2", "ones128", "maskd",
}


def build_nc(cfg: Cfg, sample_map):
    nc = bacc.Bacc(
        "TRN2", target_bir_lowering=False, debug=False,
        num_devices=2 * cfg.n_pairs,
    )
    ins = {}
    for name, arr in sample_map.items():
        dt_ = F32R if name in _F32R_INPUTS else F32
        ins[name] = nc.dram_tensor(
            name, list(arr.shape), dt_, kind="ExternalInput"
        ).ap()
    outs = {
        "outT": nc.dram_tensor(
            "outT", [2, cfg.E, cfg.SH], F32, kind="ExternalOutput"
        ).ap()
    }
    with tile.TileContext(nc) as tc:
        block_kernel(tc, cfg, ins, outs)
    nc.compile()
    return nc


_CACHE = {}


def _get_nc(cfg: Cfg, sample_map):
    if cfg not in _CACHE:
        _CACHE[cfg] = build_nc(cfg, sample_map)
    return _CACHE[cfg]


def assemble_output(cfg: Cfg, results):
    """results: list of per-core output dicts -> full [B, S, E]."""
    out = np.empty((cfg.B, cfg.S, cfg.E), np.float32)
    for p in range(cfg.n_pairs):
        oT = results[2 * p]["outT"]  # [2, E, SH]
        out[p] = np.concatenate([oT[0], oT[1]], axis=1).T
    return out


class Runner:
    """Cached PJRT runner (mirrors bass2jax.run_bass_via_pjrt multi-core
    path, but keeps the jitted executable and device-resident inputs so
    repeated calls don't re-trace / re-transfer)."""

    def __init__(self, nc, n_cores):
        import jax
        from jax.sharding import Mesh, PartitionSpec
        from jax.experimental.shard_map import shard_map
        from concourse import bass2jax, mybir as mb

        bass2jax.install_neuronx_cc_hook()
        self.nc = nc
        self.n_cores = n_cores
        partition_name = (
            nc.partition_id_tensor.name if nc.partition_id_tensor else None
        )
        in_names, out_names, out_avals, zero_outs = [], [], [], []
        for alloc in nc.m.functions[0].allocations:
            if not isinstance(alloc, mb.MemoryLocationSet):
                continue
            name = alloc.memorylocations[0].name
            if alloc.kind == "ExternalInput":
                if name != partition_name:
                    in_names.append(name)
            elif alloc.kind == "ExternalOutput":
                shape = tuple(alloc.tensor_shape)
                dtype = mb.dt.np(alloc.dtype)
                out_names.append(name)
                out_avals.append(jax.core.ShapedArray(shape, dtype))
                zero_outs.append(np.zeros(shape, dtype))
        self.in_names = list(in_names)
        self.out_names = out_names
        self.out_avals = out_avals
        self.zero_outs = zero_outs
        n_params = len(self.in_names)
        all_in = list(self.in_names) + list(out_names)
        if partition_name is not None:
            all_in.append(partition_name)
        donate = tuple(range(n_params, n_params + len(out_names)))

        def _body(*args):
            operands = list(args)
            if partition_name is not None:
                operands.append(bass2jax.partition_id_tensor())
            outs = bass2jax._bass_exec_p.bind(
                *operands,
                out_avals=tuple(out_avals),
                in_names=tuple(all_in),
                out_names=tuple(out_names),
                lowering_input_output_aliases=(),
                sim_require_finite=True,
                sim_require_nnan=True,
                nc=nc,
            )
            return tuple(outs)

        devices = jax.devices()[:n_cores]
        self.mesh = Mesh(np.asarray(devices), ("core",))
        in_specs = (PartitionSpec("core"),) * (n_params + len(out_names))
        out_specs = (PartitionSpec("core"),) * len(out_names)
        self.sharded = jax.jit(
            shard_map(
                _body, mesh=self.mesh, in_specs=in_specs, out_specs=out_specs,
                check_rep=False,
            ),
            donate_argnums=donate,
            keep_unused=True,
        )
        self._jax = jax

    def concat_inputs(self, in_maps):
        return [
            np.concatenate(
                [np.asarray(in_maps[c][n]) for c in range(self.n_cores)], axis=0
            )
            for n in self.in_names
        ]

    def fresh_zeros(self):
        return [
            np.zeros((self.n_cores * z.shape[0], *z.shape[1:]), z.dtype)
            for z in self.zero_outs
        ]

    def run(self, concat_in, zeros=None):
        if zeros is None:
            zeros = self.fresh_zeros()
        out_arrs = self.sharded(*concat_in, *zeros)
        return [
            {
                name: np.asarray(out_arrs[i]).reshape(
                    self.n_cores, *self.out_avals[i].shape
                )[c]
                for i, name in enumerate(self.out_names)
            }
            for c in range(self.n_cores)
        ]


_RUNNER = {}


def get_runner(cfg: Cfg, sample_map):
    if cfg not in _RUNNER:
        _RUNNER[cfg] = Runner(_get_nc(cfg, sample_map), 2 * cfg.n_pairs)
    return _RUNNER[cfg]


def make_in_maps(cfg: Cfg, inputs):
    x = np.asarray(inputs["x"], np.float32)
    td = int(np.asarray(inputs["transition_dim"]))
    return prep_inputs(
        cfg, x, td,
        np.asarray(inputs["ln1_g"], np.float32),
        np.asarray(inputs["ln1_b"], np.float32),
        np.asarray(inputs["ln2_g"], np.float32),
        np.asarray(inputs["ln2_b"], np.float32),
        np.asarray(inputs["w_qkv"], np.float32),
        np.asarray(inputs["b_qkv"], np.float32),
        np.asarray(inputs["w_o"], np.float32),
        np.asarray(inputs["b_o"], np.float32),
        np.asarray(inputs["w_fc1"], np.float32),
        np.asarray(inputs["b_fc1"], np.float32),
        np.asarray(inputs["w_fc2"], np.float32),
        np.asarray(inputs["b_fc2"], np.float32),
    )


def kernel(**inputs) -> np.ndarray:
    cfg = Cfg()
    in_maps = make_in_maps(cfg, inputs)
    runner = get_runner(cfg, in_maps[0])
    results = runner.run(runner.concat_inputs(in_maps))
    return assemble_output(cfg, results)



# revision 35
# speedup vs baseline: 2.7905x; 2.7905x over previous
"""Dense transformer block (nn_Block_87127706566879) on 8 TRN2 NeuronCores.

Sharding: DP over batch (4 pairs) x Megatron TP=2 within each core pair.
Each core handles one batch element, 8 of 16 heads, and half of the MLP
FF dim. One AllReduce (per core pair) after the attention out-proj and
one after fc2, both split into two sequence-halves so collectives
overlap compute.

The residual stream lives TRANSPOSED on chip ([E, S], e on partitions).
LayerNorm stats (over e = partition dim) are computed with ones-vector
matmuls on the PE; per-s stats are broadcast back across partitions with
a K=1 ones matmul. LN gains/biases are folded into the following matmul
weights host-side; residual scale factors (x/4, w_o/2, b_fc2/2, ...) are
folded so the two AllReduces reconstruct exact sums with no extra
elementwise passes.

Matmuls run in float32r (fp32 storage, ~tf32 precision, full PE rate at
free-dim >= 256). Attention: scores are computed transposed
(S^T = [k, q], keys stationary, two heads row-packed via tile_position);
softmax exp runs on ACT with the periodic (k % td == td-1) mask folded
into the per-partition bias and the 1/sqrt(dh) scale folded into the
activation scale; causal masking of diagonal tiles is a 0/1 mask
multiply on the DVE. The AV matmul appends a ones column to V
([V | 1], M=65) so the softmax denominator accumulates for free in PSUM
row 64; normalization happens once on the (much smaller) attn output.
"""

import math
from contextlib import ExitStack
from dataclasses import dataclass

import numpy as np

import concourse.bass as bass
import concourse.tile as tile
from concourse import bacc, mybir
from concourse._compat import with_exitstack

F32 = mybir.dt.float32
F32R = mybir.dt.float32r
BF = mybir.dt.bfloat16
AF = mybir.ActivationFunctionType
OP = mybir.AluOpType
NEG = -1e30


@dataclass(frozen=True)
class Cfg:
    B: int = 4
    S: int = 2048
    E: int = 1024
    H: int = 16
    FF: int = 4096
    n_pairs: int = 4  # cores = 2 * n_pairs
    gelu_exact: bool = True  # False: x*sigmoid(1.702x) (CoreSim lacks Gelu)

    @property
    def Dh(self):
        return self.E // self.H

    @property
    def HL(self):
        return self.H // 2  # heads per core

    @property
    def HP(self):
        return self.HL // 2  # head pairs per core

    @property
    def OL(self):
        return self.HL * self.Dh  # attn out dims per core

    @property
    def FL(self):
        return self.FF // 2  # ff dims per core

    @property
    def ET(self):
        return self.E // 128

    @property
    def ST(self):
        return self.S // 128

    @property
    def SH(self):
        return self.S // 2  # sequence half

    @property
    def SC(self):
        return self.SH // 512  # 512-chunks per half

    @property
    def C4(self):
        return self.S // 512  # q-chunks

    @property
    def FT(self):
        return self.FL // 128

    @property
    def OT(self):
        return self.OL // 128  # attn o-tiles (= head pairs)


@with_exitstack
def block_kernel(ctx: ExitStack, tc: tile.TileContext, cfg: Cfg, ins, outs):
    nc = tc.nc
    ET, SH, SC, C4, FT, HP, OT = (
        cfg.ET, cfg.SH, cfg.SC, cfg.C4, cfg.FT, cfg.HP, cfg.OT
    )
    S, E, OL = cfg.S, cfg.E, cfg.OL
    ST = cfg.ST
    eps1 = 1e-5 / 16.0  # LN1 runs on x/4
    eps2 = 1e-5 / 4.0   # LN2 runs on x2/2
    groups = [[2 * p, 2 * p + 1] for p in range(cfg.n_pairs)]

    cst = ctx.enter_context(tc.tile_pool(name="cst", bufs=1))

    def load_const(name, dt_):
        t = cst.tile(list(ins[name].shape), dt_, tag=name)
        nc.sync.dma_start(t[:], ins[name])
        return t

    onesrow = load_const("onesrow", F32R)      # [1, 512]
    onehot2 = load_const("onehot2", F32R)      # [2, 128]
    ones128 = load_const("ones128", F32R)      # [128, 8] (col 0: stats lhsT)
    ones128b = load_const("ones128b", BF)      # [128, 8] bf16 (LN2 stats lhsT)
    maskd = load_const("maskd", F32R)          # [128, 128] 0/1 lower-tri (k<=q)
    pbias = load_const("pbias", F32)           # [128, ST] periodic -1e30 bias
    bqk = load_const("bqk", F32)               # [128, 2*OT]
    bv = load_const("bv", F32R)                # [1, OL]
    bo_q = load_const("bo_q", F32R)            # [1, E]
    bfc1 = load_const("bfc1", F32)             # [128, FT]
    bfc2_q = load_const("bfc2_q", F32R)        # [1, E]


    def ln_stats_apply(pools, x_tiles, sc_off, eps, out_tiles, out_off,
                       apply_tiles=None, ap_off=None, ones_lhs=None):
        """LN over the partition (e) dim for one 512-col chunk of x^T.

        Stats come from x_tiles (f32r-viewable); the normalization is
        applied to apply_tiles (defaults to x_tiles; may alias out_tiles
        for in-place).
        """
        ps_stat, ps_bb, sb_small, sb_big, sb_bc = pools
        if apply_tiles is None:
            apply_tiles, ap_off = x_tiles, sc_off
        if ones_lhs is None:
            ones_lhs = ones128
        n = 512
        psum_s = ps_stat.tile([1, n], F32, tag="sum", name="sum")
        psum_q = ps_stat.tile([1, n], F32, tag="sq", name="sq")
        def dve_view(ap):
            return ap.bitcast(F32) if ap.dtype == F32R else ap

        for et in range(ET):
            xr = x_tiles[et][:, sc_off : sc_off + n]
            sq = sb_big.tile([128, n], F32R, tag="sqt", name="sqt")
            nc.vector.tensor_mul(sq[:], dve_view(xr), dve_view(xr))
            nc.tensor.matmul(
                psum_s[:], lhsT=ones_lhs[:, 0:1], rhs=xr,
                start=(et == 0), stop=(et == ET - 1),
            )
            nc.tensor.matmul(
                psum_q[:], lhsT=ones128[:, 0:1], rhs=sq[:],
                start=(et == 0), stop=(et == ET - 1),
            )
        inv_e = 1.0 / cfg.E
        m = sb_small.tile([1, n], F32R, tag="m", name="m")
        nc.vector.tensor_scalar(m[:], psum_s[:], inv_e, None, OP.mult)
        var = sb_small.tile([1, n], F32, tag="var", name="var")
        nc.vector.tensor_scalar(var[:], psum_q[:], inv_e, None, OP.mult)
        t1 = sb_small.tile([1, n], F32, tag="t1", name="t1")
        nc.vector.tensor_mul(t1[:], m[:].bitcast(F32), m[:].bitcast(F32))
        nc.vector.tensor_sub(var[:], var[:], t1[:])
        nc.vector.tensor_scalar(var[:], var[:], eps, None, OP.add)
        # rstd = rsqrt(var): ACT sqrt + DVE recip + one Newton step
        # (ACT sqrt has a loose ULP budget).
        nc.scalar.sqrt(t1[:], var[:])
        y = sb_small.tile([1, n], F32, tag="y", name="y")
        nc.vector.reciprocal(y[:], t1[:])
        nc.vector.tensor_mul(t1[:], y[:], y[:])
        nc.vector.tensor_mul(t1[:], t1[:], var[:])
        nc.vector.tensor_scalar(t1[:], t1[:], -0.5, 1.5, OP.mult, OP.add)
        rstd = sb_small.tile([1, n], F32R, tag="rstd", name="rstd")
        nc.vector.tensor_mul(rstd[:], y[:], t1[:])
        # broadcast m, rstd across partitions via K=1 ones matmul
        pm = ps_bb.tile([128, n], F32, tag="pm", name="pm")
        nc.tensor.matmul(pm[:], lhsT=onesrow[:, 0:128], rhs=m[:], start=True, stop=True)
        pr = ps_bb.tile([128, n], F32, tag="pr", name="pr")
        nc.tensor.matmul(pr[:], lhsT=onesrow[:, 0:128], rhs=rstd[:], start=True, stop=True)
        m_b = sb_bc.tile([128, n], F32, tag="m_b", name="m_b")
        nc.vector.tensor_copy(m_b[:], pm[:])
        r_b = sb_bc.tile([128, n], F32, tag="r_b", name="r_b")
        nc.vector.tensor_copy(r_b[:], pr[:])
        for et in range(ET):
            src_ap = apply_tiles[et][:, ap_off : ap_off + n]
            out_ap = out_tiles[et][:, out_off : out_off + n]
            nc.vector.tensor_sub(out_ap, dve_view(src_ap), m_b[:])
            nc.vector.tensor_mul(out_ap, out_ap, r_b[:])

    # ---------------- LN1 + QKV (qkT/VO persist into attention) ----------------
    qk_vo_stack = ExitStack()
    qk_pool = qk_vo_stack.enter_context(tc.tile_pool(name="qk", bufs=1))
    qkT = [qk_pool.tile([128, S], F32R, tag=f"qkT{i}", name=f"qkT{i}") for i in range(2 * OT)]
    vo_pool = qk_vo_stack.enter_context(tc.tile_pool(name="vo", bufs=1))
    VO = [vo_pool.tile([128, cfg.HL * 65], F32R, tag=f"vo{i}", name=f"vo{i}") for i in range(ST)]
    with (
        tc.tile_pool(name="xq_sb", bufs=1) as xq_pool,
        tc.tile_pool(name="wqk", bufs=1) as wqk_pool,
        tc.tile_pool(name="wv", bufs=1) as wv_pool,
        tc.tile_pool(name="ps_qk", bufs=2, space="PSUM") as ps_qk,
        tc.tile_pool(name="ps_v", bufs=2, space="PSUM") as ps_v,
        tc.tile_pool(name="ps_stat", bufs=1, space="PSUM") as ps_stat,
        tc.tile_pool(name="ps_bb", bufs=1, space="PSUM") as ps_bb,
        tc.tile_pool(name="sb_small", bufs=1) as sb_small,
        tc.tile_pool(name="sb_big", bufs=2) as sb_big,
        tc.tile_pool(name="sb_bc", bufs=1) as sb_bc,
    ):
        ln_pools = (ps_stat, ps_bb, sb_small, sb_big, sb_bc)
        wqk_sb = wqk_pool.tile([128, ET, 2 * OL], F32R, tag="wqk", name="wqk")
        nc.sync.dma_start(
            wqk_sb[:], ins["wqkT"].rearrange("(et p) o -> p et o", p=128)
        )
        wv_sb = wv_pool.tile([128, ET, OL], F32R, tag="wv", name="wv")
        nc.sync.dma_start(
            wv_sb[:], ins["wvT"].rearrange("(et p) o -> p et o", p=128)
        )
        for half in range(2):
            hoff = half * SH
            for sc in range(SC):
                coff = sc * 512
                xq_sb = [
                    xq_pool.tile([128, 512], F32R, tag=f"xq{et}", name=f"xq{et}")
                    for et in range(ET)
                ]
                for et in range(ET):
                    nc.sync.dma_start(
                        xq_sb[et][:],
                        ins["xq"][
                            et * 128 : (et + 1) * 128,
                            hoff + coff : hoff + coff + 512,
                        ],
                    )
                xn1 = xq_sb  # LN1 applied in place
                ln_stats_apply(ln_pools, xq_sb, 0, eps1, xn1, 0)
                # Q,K projections: out qkT [o, s] (w stationary)
                for ot in range(2 * OT):
                    psum = ps_qk.tile([128, 512], F32, tag="qk", name="qk")
                    for et in range(ET):
                        nc.tensor.matmul(
                            psum[:],
                            lhsT=wqk_sb[:, et, ot * 128 : (ot + 1) * 128],
                            rhs=xn1[et][:],
                            start=(et == 0),
                            stop=(et == ET - 1),
                        )
                    nc.vector.tensor_scalar(
                        qkT[ot][:, hoff + coff : hoff + coff + 512],
                        psum[:],
                        bqk[:, ot : ot + 1],
                        None,
                        OP.add,
                    )
                # V projection: out V [s, o_v] (xn1 stationary), bias preloaded
                for stl in range(4):  # s-tiles within this 512-chunk
                    st = (hoff + coff) // 128 + stl
                    psum = ps_v.tile([128, OL], F32, tag="v", name="v")
                    # bias broadcast preload: out[sp, o] = 1 * bv[o]
                    nc.tensor.matmul(
                        psum[:, 0:OL], lhsT=onesrow[:, 0:128], rhs=bv[:],
                        start=True, stop=False,
                    )
                    for et in range(ET):
                        nc.tensor.matmul(
                            psum[:],
                            lhsT=xn1[et][:, stl * 128 : (stl + 1) * 128],
                            rhs=wv_sb[:, et],
                            start=False,
                            stop=(et == ET - 1),
                        )
                    for h in range(cfg.HL):
                        nc.vector.tensor_copy(
                            VO[st][:, h * 65 : h * 65 + 64],
                            psum[:, h * 64 : (h + 1) * 64],
                        )
                    nc.vector.tensor_copy(
                        VO[st][:, 64 :: 65], ones128[:, 0 : cfg.HL]
                    )

    # ------------- attention + out-proj + AR1 (interleaved) -------------
    # AR1 for sequence-half 0 is issued right after attention chunks 0-1 +
    # out-proj half 0, so the collective flies while attention chunks 2-3
    # (the expensive ones) compute.
    ar1_in = [nc.dram_tensor(f"ar1_in{h}", [E, SH], BF) for h in range(2)]
    ar1_out = [nc.dram_tensor(f"ar1_out{h}", [E, SH], BF) for h in range(2)]
    at_stack = ExitStack()
    at_pool = at_stack.enter_context(tc.tile_pool(name="attnT", bufs=1))
    attnT = [at_pool.tile([128, S], F32R, tag=f"at{i}", name=f"at{i}") for i in range(OT)]
    with (
        tc.tile_pool(name="pt", bufs=3) as pt_pool,
        tc.tile_pool(name="den", bufs=2) as den_pool,
        tc.tile_pool(name="wo", bufs=1) as wo_pool,
        tc.tile_pool(name="xqs", bufs=4) as xqs_pool,
        tc.tile_pool(name="arin", bufs=4) as arin_pool,
        tc.tile_pool(name="ps_sc", bufs=2, space="PSUM") as ps_sc,
        tc.tile_pool(name="ps_av", bufs=1, space="PSUM") as ps_av,
        tc.tile_pool(name="ps_bc", bufs=1, space="PSUM") as ps_bc,
        tc.tile_pool(name="ps_o", bufs=1, space="PSUM") as ps_o,
    ):
        bo_q = wo_pool.tile([1, E], F32R, tag="bo_q", name="bo_q")
        nc.sync.dma_start(bo_q[:], ins["bo_q"])
        wo_sb = wo_pool.tile([128, OT, E], F32R, tag="wo", name="wo")
        nc.sync.dma_start(
            wo_sb[:], ins["woT_q"].rearrange("(ot p) e -> p ot e", p=128)
        )

        def out_proj_half(half):
            hoff = half * SH
            for et in range(ET):
                for sc in range(SC):
                    coff = hoff + sc * 512
                    psum = ps_o.tile([128, 512], F32, tag="o", name="o")
                    nc.tensor.matmul(
                        psum[:],
                        lhsT=bo_q[:, et * 128 : (et + 1) * 128],
                        rhs=onesrow[:],
                        start=True,
                        stop=False,
                    )
                    for ot in range(OT):
                        nc.tensor.matmul(
                            psum[:],
                            lhsT=wo_sb[:, ot, et * 128 : (et + 1) * 128],
                            rhs=attnT[ot][:, coff : coff + 512],
                            start=False,
                            stop=(ot == OT - 1),
                        )
                    xqs = xqs_pool.tile([128, 512], F32, tag="xqs", name="xqs")
                    nc.sync.dma_start(
                        xqs[:],
                        ins["xq"].bitcast(F32)[
                            et * 128 : (et + 1) * 128, coff : coff + 512
                        ],
                    )
                    arin = arin_pool.tile([128, 512], BF, tag="arin", name="arin")
                    nc.vector.tensor_add(arin[:], psum[:], xqs[:])
                    nc.sync.dma_start(
                        ar1_in[half][
                            et * 128 : (et + 1) * 128, sc * 512 : sc * 512 + 512
                        ],
                        arin[:],
                    )

        def ar1_issue(half):
            nc.gpsimd.collective_compute(
                "AllReduce",
                OP.add,
                replica_groups=groups,
                ins=[ar1_in[half][:]],
                outs=[ar1_out[half][:]],
            )

        scale = 1.0 / math.sqrt(cfg.Dh)
        for c in range(C4):
            kmax = 4 * c + 4
            for hp in range(HP):
                av = [ps_av.tile([65, 512], F32, tag=f"av{h01}", name=f"av{h01}") for h01 in range(2)]
                for i in range(kmax):
                    psc = ps_sc.tile([128, 1024], F32, tag="sc", name="sc")
                    for h01 in range(2):
                        po = h01 * 64
                        nc.tensor.matmul(
                            psc[:, h01 * 512 : (h01 + 1) * 512],
                            lhsT=qkT[OT + hp][po : po + 64, i * 128 : (i + 1) * 128],
                            rhs=qkT[hp][po : po + 64, c * 512 : (c + 1) * 512],
                            start=True,
                            stop=True,
                            tile_position=(po, 0),
                        )
                    pt = pt_pool.tile([128, 1024], F32R, tag="pt", name="pt")
                    nc.scalar.activation(
                        pt[:], psc[:], AF.Exp,
                        bias=pbias[:, i : i + 1], scale=scale,
                    )
                    r = i - 4 * c
                    if r >= 0:
                        for h01 in range(2):
                            if r > 0:
                                zsl = pt[:, h01 * 512 : h01 * 512 + r * 128]
                                nc.vector.tensor_scalar(
                                    zsl, zsl, 0.0, None, OP.mult
                                )
                            sl = slice(h01 * 512 + r * 128, h01 * 512 + (r + 1) * 128)
                            nc.vector.tensor_mul(pt[:, sl], pt[:, sl], maskd[:])
                    for h01 in range(2):
                        hloc = 2 * hp + h01
                        nc.tensor.matmul(
                            av[h01][:],
                            lhsT=VO[i][:, hloc * 65 : (hloc + 1) * 65],
                            rhs=pt[:, h01 * 512 : (h01 + 1) * 512],
                            start=(i == 0),
                            stop=(i == kmax - 1),
                        )
                dens = [
                    den_pool.tile([1, 512], F32R, tag=f"den{h01}", name=f"den{h01}")
                    for h01 in range(2)
                ]
                with nc.allow_low_precision(reason="f32r rounding for matmul rhs"):
                    for h01 in range(2):
                        nc.vector.reciprocal(dens[h01][:], av[h01][64:65, :])
                # assemble [2, 512] (DMA can place row 1; DVE cannot)
                den2 = den_pool.tile([2, 512], F32R, tag="den2", name="den2")
                for h01 in range(2):
                    nc.sync.dma_start(den2[h01 : h01 + 1, :], dens[h01][:])
                pbc = ps_bc.tile([128, 512], F32, tag="bc", name="bc")
                nc.tensor.matmul(
                    pbc[:], lhsT=onehot2[:], rhs=den2[:], start=True, stop=True
                )
                sbc = den_pool.tile([128, 512], F32, tag="sbc", name="sbc")
                nc.vector.tensor_copy(sbc[:], pbc[:])
                for h01 in range(2):
                    nc.vector.tensor_mul(
                        attnT[hp][h01 * 64 : (h01 + 1) * 64, c * 512 : (c + 1) * 512],
                        av[h01][0:64, :],
                        sbc[h01 * 64 : (h01 + 1) * 64, :],
                    )
            if c == 1:
                out_proj_half(0)
                ar1_issue(0)
        out_proj_half(1)

    at_stack.close()   # attnT no longer needed
    qk_vo_stack.close()  # qkT/VO no longer needed

    # ---------------- LN2 + MLP + AR2 ----------------
    ar2_in = [nc.dram_tensor(f"ar2_in{h}", [E, SH], BF) for h in range(2)]
    ar2_out = [nc.dram_tensor(f"ar2_out{h}", [E, SH], BF) for h in range(2)]
    with (
        tc.tile_pool(name="x2b", bufs=2) as x2b_pool,
        tc.tile_pool(name="x2res", bufs=3) as x2res_pool,
        tc.tile_pool(name="xn2", bufs=1) as xn2_pool,
        tc.tile_pool(name="ht", bufs=1) as ht_pool,
        tc.tile_pool(name="w1", bufs=3) as w1_pool,
        tc.tile_pool(name="w2", bufs=2) as w2_pool,
        tc.tile_pool(name="fin", bufs=2) as fin_pool,
        tc.tile_pool(name="ps_f1", bufs=2, space="PSUM") as ps_f1,
        tc.tile_pool(name="ps_f2", bufs=2, space="PSUM") as ps_f2,
        tc.tile_pool(name="ps_stat", bufs=1, space="PSUM") as ps_stat,
        tc.tile_pool(name="ps_bb", bufs=1, space="PSUM") as ps_bb,
        tc.tile_pool(name="sb_small", bufs=1) as sb_small,
        tc.tile_pool(name="sb_big", bufs=2) as sb_big,
        tc.tile_pool(name="sb_bc", bufs=1) as sb_bc,
    ):
        ln_pools = (ps_stat, ps_bb, sb_small, sb_big, sb_bc)
        bfc2_q = w2_pool.tile([1, E], F32R, tag="bfc2_q", name="bfc2_q")
        nc.sync.dma_start(bfc2_q[:], ins["bfc2_q"])

        def ar2_issue(half):
            nc.gpsimd.collective_compute(
                "AllReduce",
                OP.add,
                replica_groups=groups,
                ins=[ar2_in[half][:]],
                outs=[ar2_out[half][:]],
            )
            nc.sync.dma_start(outs["outT"][half], ar2_out[half][:])

        def mlp_ln(half):
            xn2 = [xn2_pool.tile([128, SH], F32R, tag=f"xn2{et}", name=f"xn2{et}") for et in range(ET)]
            for sc in range(SC):
                coff = sc * 512
                x2b = [
                    x2b_pool.tile([128, 512], BF, tag=f"x2b{et}", name=f"x2b{et}")
                    for et in range(ET)
                ]
                for et in range(ET):
                    nc.sync.dma_start(
                        x2b[et][:],
                        ar1_out[half][
                            et * 128 : (et + 1) * 128, coff : coff + 512
                        ],
                    )
                ln_stats_apply(ln_pools, x2b, 0, eps2, xn2, coff,
                               ones_lhs=ones128b)
            return xn2

        def mlp_ffn(half, xn2):
            ht_tiles = []
            for ft in range(FT):
                w1t = w1_pool.tile([128, ET, 128], F32R, tag="w1", name="w1")
                nc.sync.dma_start(
                    w1t[:],
                    ins["wfc1T"][:, ft * 128 : (ft + 1) * 128].rearrange(
                        "(et p) f -> p et f", p=128
                    ),
                )
                ht = ht_pool.tile([128, SH], F32R, tag=f"ht{ft}", name=f"ht{ft}")
                for sc in range(SC):
                    psum = ps_f1.tile([128, 512], F32, tag="f1", name="f1")
                    for et in range(ET):
                        nc.tensor.matmul(
                            psum[:],
                            lhsT=w1t[:, et],
                            rhs=xn2[et][:, sc * 512 : (sc + 1) * 512],
                            start=(et == 0),
                            stop=(et == ET - 1),
                        )
                    hsl = ht[:, sc * 512 : (sc + 1) * 512]
                    if cfg.gelu_exact:
                        nc.scalar.activation(
                            hsl, psum[:], AF.Gelu,
                            bias=bfc1[:, ft : ft + 1], scale=1.0,
                        )
                    else:
                        tg = fin_pool.tile([128, 512], F32, tag="tg", name="tg")
                        nc.vector.tensor_scalar(
                            tg[:], psum[:], bfc1[:, ft : ft + 1], None, OP.add
                        )
                        sg = fin_pool.tile([128, 512], F32, tag="sg", name="sg")
                        nc.scalar.activation(sg[:], tg[:], AF.Sigmoid, scale=1.702)
                        nc.vector.tensor_mul(hsl, tg[:], sg[:])
                ht_tiles.append(ht)
            for et in range(ET):
                w2t = w2_pool.tile([128, FT, 128], F32R, tag="w2", name="w2")
                nc.sync.dma_start(
                    w2t[:],
                    ins["wfc2T"][:, et * 128 : (et + 1) * 128].rearrange(
                        "(ft p) e -> p ft e", p=128
                    ),
                )
                fin = fin_pool.tile([128, SH], BF, tag="fin", name="fin")
                for sc in range(SC):
                    psum = ps_f2.tile([128, 512], F32, tag="f2", name="f2")
                    nc.tensor.matmul(
                        psum[:],
                        lhsT=bfc2_q[:, et * 128 : (et + 1) * 128],
                        rhs=onesrow[:],
                        start=True,
                        stop=False,
                    )
                    for ft in range(FT):
                        nc.tensor.matmul(
                            psum[:],
                            lhsT=w2t[:, ft],
                            rhs=ht_tiles[ft][:, sc * 512 : (sc + 1) * 512],
                            start=False,
                            stop=(ft == FT - 1),
                        )
                    x2res = x2res_pool.tile([128, 512], BF, tag="x2res", name="x2res")
                    nc.sync.dma_start(
                        x2res[:],
                        ar1_out[half][
                            et * 128 : (et + 1) * 128,
                            sc * 512 : (sc + 1) * 512,
                        ],
                    )
                    nc.vector.tensor_add(
                        fin[:, sc * 512 : (sc + 1) * 512], psum[:], x2res[:]
                    )
                nc.sync.dma_start(
                    ar2_in[half][et * 128 : (et + 1) * 128, :], fin[:]
                )

        xn2_0 = mlp_ln(0)
        ar1_issue(1)   # late issue: keeps h0 LN reads off the AR1h1 wait
        mlp_ffn(0, xn2_0)
        xn2_1 = mlp_ln(1)
        ar2_issue(0)   # late issue: keeps h1 LN reads off the AR2h0 wait
        mlp_ffn(1, xn2_1)
        ar2_issue(1)


# ---------------------------------------------------------------------------
# host side
# ---------------------------------------------------------------------------

def prep_inputs(cfg: Cfg, x, td, ln1_g, ln1_b, ln2_g, ln2_b, w_qkv, b_qkv,
                w_o, b_o, w_fc1, b_fc1, w_fc2, b_fc2):
    """Build the per-core input maps (numpy, fp32)."""
    E, H, FL, OL, HL = cfg.E, cfg.H, cfg.FL, cfg.OL, cfg.HL
    f4 = np.float32
    asc = np.ascontiguousarray

    wq, wk, wv = w_qkv[0:E], w_qkv[E : 2 * E], w_qkv[2 * E : 3 * E]
    bq, bk, bvv = b_qkv[0:E], b_qkv[E : 2 * E], b_qkv[2 * E : 3 * E]

    shared = {}
    shared["onesrow"] = np.ones((1, 512), f4)
    oh = np.zeros((2, 128), f4)
    oh[0, 0:64] = 1.0
    oh[1, 64:128] = 1.0
    shared["onehot2"] = oh
    shared["ones128"] = np.ones((128, 8), f4)
    import ml_dtypes

    shared["ones128b"] = np.ones((128, 8), ml_dtypes.bfloat16)
    k_idx = np.arange(128)
    shared["maskd"] = asc((k_idx[:, None] <= k_idx[None, :]).astype(f4))
    pb = np.zeros((128, cfg.ST), f4)
    for i in range(cfg.ST):
        kabs = i * 128 + k_idx
        pb[(kabs % td) == (td - 1), i] = NEG
    shared["pbias"] = pb
    shared["bo_q"] = asc(0.25 * b_o[None, :].astype(f4))
    shared["bfc2_q"] = asc(0.5 * b_fc2[None, :].astype(f4))

    per_tp = []
    for tp in range(2):
        o_sl = slice(tp * OL, (tp + 1) * OL)
        f_sl = slice(tp * FL, (tp + 1) * FL)
        d = {}
        wqk = np.concatenate([wq[o_sl], wk[o_sl]], axis=0)  # [2*OL, E]
        d["wqkT"] = asc((wqk * ln1_g[None, :]).T.astype(f4))  # [E, 2*OL]
        bqk_full = (
            np.concatenate([bq[o_sl], bk[o_sl]]) + wqk @ ln1_b
        ).astype(f4)  # [2*OL]
        d["bqk"] = asc(bqk_full.reshape(2 * cfg.OT, 128).T)  # [128, 2*OT]
        d["wvT"] = asc((wv[o_sl] * ln1_g[None, :]).T.astype(f4))  # [E, OL]
        d["bv"] = asc((bvv[o_sl] + wv[o_sl] @ ln1_b)[None, :].astype(f4))  # [1, OL]
        d["woT_q"] = asc(0.5 * w_o[:, o_sl].T.astype(f4))  # [OL, E]
        d["wfc1T"] = asc((w_fc1[f_sl] * ln2_g[None, :]).T.astype(f4))  # [E, FL]
        d["bfc1"] = asc(
            (b_fc1[f_sl] + w_fc1[f_sl] @ ln2_b).astype(f4).reshape(cfg.FT, 128).T
        )  # [128, FT]
        d["wfc2T"] = asc(w_fc2[:, f_sl].T.astype(f4))  # [FL, E]
        per_tp.append(d)

    in_maps = []
    for c in range(2 * cfg.n_pairs):
        p, tp = c // 2, c % 2
        m = dict(shared)
        m.update(per_tp[tp])
        m["xq"] = asc(0.25 * x[p].T.astype(f4))  # [E, S]
        in_maps.append(m)
    return in_maps


_F32R_INPUTS = {
    "xq", "wqkT", "wvT", "bv", "woT_q", "bo_q", "wfc1T", "wfc2T", "bfc2_q",
    "onesrow", "onehot2", "ones128", "maskd",
}
_BF16_INPUTS = {"ones128b"}


def build_nc(cfg: Cfg, sample_map):
    nc = bacc.Bacc(
        "TRN2", target_bir_lowering=False, debug=False,
        num_devices=2 * cfg.n_pairs,
    )
    ins = {}
    for name, arr in sample_map.items():
        dt_ = BF if name in _BF16_INPUTS else (
            F32R if name in _F32R_INPUTS else F32
        )
        ins[name] = nc.dram_tensor(
            name, list(arr.shape), dt_, kind="ExternalInput"
        ).ap()
    outs = {
        "outT": nc.dram_tensor(
            "outT", [2, cfg.E, cfg.SH], BF, kind="ExternalOutput"
        ).ap()
    }
    with tile.TileContext(nc) as tc:
        block_kernel(tc, cfg, ins, outs)
    nc.compile()
    return nc


_CACHE = {}


def _get_nc(cfg: Cfg, sample_map):
    if cfg not in _CACHE:
        _CACHE[cfg] = build_nc(cfg, sample_map)
    return _CACHE[cfg]


def assemble_output(cfg: Cfg, results):
    """results: list of per-core output dicts -> full [B, S, E]."""
    out = np.empty((cfg.B, cfg.S, cfg.E), np.float32)
    for p in range(cfg.n_pairs):
        oT = np.asarray(results[2 * p]["outT"], np.float32)  # [2, E, SH]
        out[p] = np.concatenate([oT[0], oT[1]], axis=1).T
    return out


class Runner:
    """Cached PJRT runner (mirrors bass2jax.run_bass_via_pjrt multi-core
    path, but keeps the jitted executable and device-resident inputs so
    repeated calls don't re-trace / re-transfer)."""

    def __init__(self, nc, n_cores):
        import jax
        from jax.sharding import Mesh, PartitionSpec
        from jax.experimental.shard_map import shard_map
        from concourse import bass2jax, mybir as mb

        bass2jax.install_neuronx_cc_hook()
        self.nc = nc
        self.n_cores = n_cores
        partition_name = (
            nc.partition_id_tensor.name if nc.partition_id_tensor else None
        )
        in_names, out_names, out_avals, zero_outs = [], [], [], []
        for alloc in nc.m.functions[0].allocations:
            if not isinstance(alloc, mb.MemoryLocationSet):
                continue
            name = alloc.memorylocations[0].name
            if alloc.kind == "ExternalInput":
                if name != partition_name:
                    in_names.append(name)
            elif alloc.kind == "ExternalOutput":
                shape = tuple(alloc.tensor_shape)
                dtype = mb.dt.np(alloc.dtype)
                out_names.append(name)
                out_avals.append(jax.core.ShapedArray(shape, dtype))
                zero_outs.append(np.zeros(shape, dtype))
        self.in_names = list(in_names)
        self.out_names = out_names
        self.out_avals = out_avals
        self.zero_outs = zero_outs
        n_params = len(self.in_names)
        all_in = list(self.in_names) + list(out_names)
        if partition_name is not None:
            all_in.append(partition_name)
        donate = tuple(range(n_params, n_params + len(out_names)))

        def _body(*args):
            operands = list(args)
            if partition_name is not None:
                operands.append(bass2jax.partition_id_tensor())
            outs = bass2jax._bass_exec_p.bind(
                *operands,
                out_avals=tuple(out_avals),
                in_names=tuple(all_in),
                out_names=tuple(out_names),
                lowering_input_output_aliases=(),
                sim_require_finite=True,
                sim_require_nnan=True,
                nc=nc,
            )
            return tuple(outs)

        devices = jax.devices()[:n_cores]
        self.mesh = Mesh(np.asarray(devices), ("core",))
        in_specs = (PartitionSpec("core"),) * (n_params + len(out_names))
        out_specs = (PartitionSpec("core"),) * len(out_names)
        self.sharded = jax.jit(
            shard_map(
                _body, mesh=self.mesh, in_specs=in_specs, out_specs=out_specs,
                check_rep=False,
            ),
            donate_argnums=donate,
            keep_unused=True,
        )
        self._jax = jax

    def concat_inputs(self, in_maps):
        return [
            np.concatenate(
                [np.asarray(in_maps[c][n]) for c in range(self.n_cores)], axis=0
            )
            for n in self.in_names
        ]

    def fresh_zeros(self):
        return [
            np.zeros((self.n_cores * z.shape[0], *z.shape[1:]), z.dtype)
            for z in self.zero_outs
        ]

    def run(self, concat_in, zeros=None):
        if zeros is None:
            zeros = self.fresh_zeros()
        out_arrs = self.sharded(*concat_in, *zeros)
        return [
            {
                name: np.asarray(out_arrs[i]).reshape(
                    self.n_cores, *self.out_avals[i].shape
                )[c]
                for i, name in enumerate(self.out_names)
            }
            for c in range(self.n_cores)
        ]


_RUNNER = {}


def get_runner(cfg: Cfg, sample_map):
    if cfg not in _RUNNER:
        _RUNNER[cfg] = Runner(_get_nc(cfg, sample_map), 2 * cfg.n_pairs)
    return _RUNNER[cfg]


def make_in_maps(cfg: Cfg, inputs):
    x = np.asarray(inputs["x"], np.float32)
    td = int(np.asarray(inputs["transition_dim"]))
    return prep_inputs(
        cfg, x, td,
        np.asarray(inputs["ln1_g"], np.float32),
        np.asarray(inputs["ln1_b"], np.float32),
        np.asarray(inputs["ln2_g"], np.float32),
        np.asarray(inputs["ln2_b"], np.float32),
        np.asarray(inputs["w_qkv"], np.float32),
        np.asarray(inputs["b_qkv"], np.float32),
        np.asarray(inputs["w_o"], np.float32),
        np.asarray(inputs["b_o"], np.float32),
        np.asarray(inputs["w_fc1"], np.float32),
        np.asarray(inputs["b_fc1"], np.float32),
        np.asarray(inputs["w_fc2"], np.float32),
        np.asarray(inputs["b_fc2"], np.float32),
    )


def kernel(**inputs) -> np.ndarray:
    cfg = Cfg()
    in_maps = make_in_maps(cfg, inputs)
    runner = get_runner(cfg, in_maps[0])
    results = runner.run(runner.concat_inputs(in_maps))
    return assemble_output(cfg, results)



# revision 58
# speedup vs baseline: 3.6245x; 1.2989x over previous
"""Dense transformer block (nn_Block_87127706566879) on 8 TRN2 NeuronCores.

Sharding: DP over batch (4 pairs) x Megatron TP=2 within each core pair.
Each core handles one batch element, 8 of 16 heads, and half of the MLP
FF dim. One AllReduce (per core pair) after the attention out-proj and
one after fc2, both split into two sequence-halves so collectives
overlap compute. The AllReduces run in bf16 (partials carry pre-scaled
residuals, so the pair sum reconstructs x/2 + branch/2 exactly up to
bf16 rounding); out-proj half 0 + its AllReduce issue mid-attention so
the collective flies under attention chunks 2-3, and each collective's
issue is deferred past the next stage's reads to keep conservative
semaphore waits off the critical path. The first x chunk loads ahead of
the big weight DMAs and LN1 input tiles are double-buffered so each
chunk's stats overlap the previous chunk's projections.

The residual stream lives TRANSPOSED on chip ([E, S], e on partitions).
LayerNorm stats (over e = partition dim) are computed with ones-vector
matmuls on the PE; per-s stats are broadcast back across partitions with
a K=1 ones matmul. LN gains/biases are folded into the following matmul
weights host-side; residual scale factors (x/4, w_o/2, b_fc2/2, ...) are
folded so the two AllReduces reconstruct exact sums with no extra
elementwise passes.

Matmuls run in float32r (fp32 storage, ~tf32 precision, full PE rate at
free-dim >= 256). Attention: scores are computed transposed
(S^T = [k, q], keys stationary, two heads row-packed via tile_position);
softmax exp runs on ACT with the periodic (k % td == td-1) mask folded
into the per-partition bias and the 1/sqrt(dh) scale folded into the
activation scale; causal masking of diagonal tiles is a 0/1 mask
multiply on the DVE. The AV matmul appends a ones column to V
([V | 1], M=65) so the softmax denominator accumulates for free in PSUM
row 64; normalization happens once on the (much smaller) attn output.
"""

import math
import os
from contextlib import ExitStack
from dataclasses import dataclass

import numpy as np

_NO_CC = bool(os.environ.get("BASS_KERNEL_NO_CC"))  # timing experiment only

import concourse.bass as bass
import concourse.tile as tile
from concourse import bacc, mybir
from concourse._compat import with_exitstack

F32 = mybir.dt.float32
F32R = mybir.dt.float32r
BF = mybir.dt.bfloat16
F8 = mybir.dt.float8e4
AF = mybir.ActivationFunctionType
OP = mybir.AluOpType
NEG = -1e30


@dataclass(frozen=True)
class Cfg:
    B: int = 4
    S: int = 2048
    E: int = 1024
    H: int = 16
    FF: int = 4096
    n_pairs: int = 4  # cores = 2 * n_pairs
    gelu_exact: bool = True  # False: x*sigmoid(1.702x) (CoreSim lacks Gelu)

    @property
    def Dh(self):
        return self.E // self.H

    @property
    def HL(self):
        return self.H // 2  # heads per core

    @property
    def HP(self):
        return self.HL // 2  # head pairs per core

    @property
    def OL(self):
        return self.HL * self.Dh  # attn out dims per core

    @property
    def FL(self):
        return self.FF // 2  # ff dims per core

    @property
    def ET(self):
        return self.E // 128

    @property
    def ST(self):
        return self.S // 128

    @property
    def SH(self):
        return self.S // 2  # sequence half

    @property
    def SC(self):
        return self.SH // 512  # 512-chunks per half

    @property
    def C4(self):
        return self.S // 512  # q-chunks

    @property
    def FT(self):
        return self.FL // 128

    @property
    def OT(self):
        return self.OL // 128  # attn o-tiles (= head pairs)


@with_exitstack
def block_kernel(ctx: ExitStack, tc: tile.TileContext, cfg: Cfg, ins, outs):
    nc = tc.nc
    ET, SH, SC, C4, FT, HP, OT = (
        cfg.ET, cfg.SH, cfg.SC, cfg.C4, cfg.FT, cfg.HP, cfg.OT
    )
    S, E, OL = cfg.S, cfg.E, cfg.OL
    ST = cfg.ST
    eps1 = 1e-5 / 16.0  # LN1 runs on x/4
    eps2 = 1e-5 / 4.0   # LN2 runs on x2/2
    groups = [[2 * p, 2 * p + 1] for p in range(cfg.n_pairs)]

    cst = ctx.enter_context(tc.tile_pool(name="cst", bufs=1))

    def load_const(name, dt_):
        t = cst.tile(list(ins[name].shape), dt_, tag=name)
        nc.sync.dma_start(t[:], ins[name])
        return t

    onesrow = load_const("onesrow", F32R)      # [1, 512]
    onehot2 = load_const("onehot2", F32R)      # [2, 128]
    ones128 = load_const("ones128", F32R)      # [128, 8] (col 0: stats lhsT)
    ones128b = load_const("ones128b", BF)      # [128, 8] bf16 (LN2 stats lhsT)
    maskd = load_const("maskd", F32R)          # [128, 128] 0/1 lower-tri (k<=q)
    pbias = load_const("pbias", F32)           # [128, ST] periodic -1e30 bias
    bqk = load_const("bqk", F32)               # [128, 2*OT]
    bv = load_const("bv", F32R)                # [1, OL]
    bo_q = load_const("bo_q", F32R)            # [1, E]
    bfc1 = load_const("bfc1", F32)             # [128, FT]
    bfc2_q = load_const("bfc2_q", F32R)        # [1, E]


    def ln_stats_apply(pools, x_tiles, sc_off, eps, out_tiles, out_off,
                       apply_tiles=None, ap_off=None, ones_lhs=None):
        """LN over the partition (e) dim for one 512-col chunk of x^T.

        Stats come from x_tiles (f32r-viewable); the normalization is
        applied to apply_tiles (defaults to x_tiles; may alias out_tiles
        for in-place).
        """
        ps_stat, ps_bb, sb_small, sb_big, sb_bc = pools
        if apply_tiles is None:
            apply_tiles, ap_off = x_tiles, sc_off
        if ones_lhs is None:
            ones_lhs = ones128
        n = 512
        psum_s = ps_stat.tile([1, n], F32, tag="sum", name="sum")
        psum_q = ps_stat.tile([1, n], F32, tag="sq", name="sq")
        def dve_view(ap):
            return ap.bitcast(F32) if ap.dtype == F32R else ap

        for et in range(ET):
            xr = x_tiles[et][:, sc_off : sc_off + n]
            sq = sb_big.tile([128, n], F32R, tag="sqt", name="sqt")
            nc.vector.tensor_mul(sq[:], dve_view(xr), dve_view(xr))
            nc.tensor.matmul(
                psum_s[:], lhsT=ones_lhs[:, 0:1], rhs=xr,
                start=(et == 0), stop=(et == ET - 1),
            )
            nc.tensor.matmul(
                psum_q[:], lhsT=ones128[:, 0:1], rhs=sq[:],
                start=(et == 0), stop=(et == ET - 1),
            )
        inv_e = 1.0 / cfg.E
        m = sb_small.tile([1, n], F32R, tag="m", name="m")
        nc.vector.tensor_scalar(m[:], psum_s[:], inv_e, None, OP.mult)
        var = sb_small.tile([1, n], F32, tag="var", name="var")
        nc.vector.tensor_scalar(var[:], psum_q[:], inv_e, None, OP.mult)
        t1 = sb_small.tile([1, n], F32, tag="t1", name="t1")
        nc.vector.tensor_mul(t1[:], m[:].bitcast(F32), m[:].bitcast(F32))
        nc.vector.tensor_sub(var[:], var[:], t1[:])
        nc.vector.tensor_scalar(var[:], var[:], eps, None, OP.add)
        # rstd = rsqrt(var): ACT sqrt + DVE recip + one Newton step
        # (ACT sqrt has a loose ULP budget).
        nc.scalar.sqrt(t1[:], var[:])
        y = sb_small.tile([1, n], F32, tag="y", name="y")
        nc.vector.reciprocal(y[:], t1[:])
        nc.vector.tensor_mul(t1[:], y[:], y[:])
        nc.vector.tensor_mul(t1[:], t1[:], var[:])
        nc.vector.tensor_scalar(t1[:], t1[:], -0.5, 1.5, OP.mult, OP.add)
        rstd = sb_small.tile([1, n], F32R, tag="rstd", name="rstd")
        nc.vector.tensor_mul(rstd[:], y[:], t1[:])
        # broadcast m, rstd across partitions via K=1 ones matmul
        pm = ps_bb.tile([128, n], F32, tag="pm", name="pm")
        nc.tensor.matmul(pm[:], lhsT=onesrow[:, 0:128], rhs=m[:], start=True, stop=True)
        pr = ps_bb.tile([128, n], F32, tag="pr", name="pr")
        nc.tensor.matmul(pr[:], lhsT=onesrow[:, 0:128], rhs=rstd[:], start=True, stop=True)
        m_b = sb_bc.tile([128, n], F32, tag="m_b", name="m_b")
        nc.vector.tensor_copy(m_b[:], pm[:])
        r_b = sb_bc.tile([128, n], F32, tag="r_b", name="r_b")
        nc.vector.tensor_copy(r_b[:], pr[:])
        for et in range(ET):
            src_ap = apply_tiles[et][:, ap_off : ap_off + n]
            out_ap = out_tiles[et][:, out_off : out_off + n]
            nc.vector.tensor_sub(out_ap, dve_view(src_ap), m_b[:])
            nc.vector.tensor_mul(out_ap, out_ap, r_b[:])

    # ---------------- LN1 + QKV (qkT/VO persist into attention) ----------------
    qk_vo_stack = ExitStack()
    qk_pool = qk_vo_stack.enter_context(tc.tile_pool(name="qk", bufs=1))
    qkT = [qk_pool.tile([128, S], F32R, tag=f"qkT{i}", name=f"qkT{i}") for i in range(2 * OT)]
    vo_pool = qk_vo_stack.enter_context(tc.tile_pool(name="vo", bufs=1))
    VO = [vo_pool.tile([128, cfg.HL * 65], F32R, tag=f"vo{i}", name=f"vo{i}") for i in range(ST)]
    with (
        tc.tile_pool(name="xq_sb", bufs=2) as xq_pool,
        tc.tile_pool(name="wqk", bufs=1) as wqk_pool,
        tc.tile_pool(name="wv", bufs=1) as wv_pool,
        tc.tile_pool(name="ps_qk", bufs=2, space="PSUM") as ps_qk,
        tc.tile_pool(name="ps_v", bufs=2, space="PSUM") as ps_v,
        tc.tile_pool(name="ps_stat", bufs=1, space="PSUM") as ps_stat,
        tc.tile_pool(name="ps_bb", bufs=1, space="PSUM") as ps_bb,
        tc.tile_pool(name="sb_small", bufs=1) as sb_small,
        tc.tile_pool(name="sb_big", bufs=2) as sb_big,
        tc.tile_pool(name="sb_bc", bufs=1) as sb_bc,
    ):
        ln_pools = (ps_stat, ps_bb, sb_small, sb_big, sb_bc)

        def load_xq_chunk(half, sc):
            hoff, coff = half * SH, sc * 512
            xq_sb = [
                xq_pool.tile([128, 512], F32R, tag=f"xq{et}", name=f"xq{et}")
                for et in range(ET)
            ]
            for et in range(ET):
                nc.sync.dma_start(
                    xq_sb[et][:],
                    ins["xq"][
                        et * 128 : (et + 1) * 128,
                        hoff + coff : hoff + coff + 512,
                    ],
                )
            return xq_sb

        # first x chunk loads ahead of the big weight DMAs (startup path)
        next_xq = load_xq_chunk(0, 0)
        wqk_sb = wqk_pool.tile([128, ET, 2 * OL], F32R, tag="wqk", name="wqk")
        nc.sync.dma_start(
            wqk_sb[:], ins["wqkT"].rearrange("(et p) o -> p et o", p=128)
        )
        wv_sb = wv_pool.tile([128, ET, OL], F32R, tag="wv", name="wv")
        nc.sync.dma_start(
            wv_sb[:], ins["wvT"].rearrange("(et p) o -> p et o", p=128)
        )
        for half in range(2):
            hoff = half * SH
            for sc in range(SC):
                coff = sc * 512
                xq_sb = next_xq
                if (half, sc) != (1, SC - 1):
                    nh, nsc = (half, sc + 1) if sc + 1 < SC else (half + 1, 0)
                    next_xq = load_xq_chunk(nh, nsc)
                xn1 = xq_sb  # LN1 applied in place
                ln_stats_apply(ln_pools, xq_sb, 0, eps1, xn1, 0)
                # Q,K projections: out qkT [o, s] (w stationary)
                for ot in range(2 * OT):
                    psum = ps_qk.tile([128, 512], F32, tag="qk", name="qk")
                    for et in range(ET):
                        nc.tensor.matmul(
                            psum[:],
                            lhsT=wqk_sb[:, et, ot * 128 : (ot + 1) * 128],
                            rhs=xn1[et][:],
                            start=(et == 0),
                            stop=(et == ET - 1),
                        )
                    nc.vector.tensor_scalar(
                        qkT[ot][:, hoff + coff : hoff + coff + 512],
                        psum[:],
                        bqk[:, ot : ot + 1],
                        None,
                        OP.add,
                    )
                # V projection: out V [s, o_v] (xn1 stationary), bias preloaded
                for stl in range(4):  # s-tiles within this 512-chunk
                    st = (hoff + coff) // 128 + stl
                    psum = ps_v.tile([128, OL], F32, tag="v", name="v")
                    # bias broadcast preload: out[sp, o] = 1 * bv[o]
                    nc.tensor.matmul(
                        psum[:, 0:OL], lhsT=onesrow[:, 0:128], rhs=bv[:],
                        start=True, stop=False,
                    )
                    for et in range(ET):
                        nc.tensor.matmul(
                            psum[:],
                            lhsT=xn1[et][:, stl * 128 : (stl + 1) * 128],
                            rhs=wv_sb[:, et],
                            start=False,
                            stop=(et == ET - 1),
                        )
                    for h in range(cfg.HL):
                        nc.vector.tensor_copy(
                            VO[st][:, h * 65 : h * 65 + 64],
                            psum[:, h * 64 : (h + 1) * 64],
                        )
                    nc.vector.tensor_copy(
                        VO[st][:, 64 :: 65], ones128[:, 0 : cfg.HL]
                    )

    # ------------- attention + out-proj + AR1 (interleaved) -------------
    # AR1 for sequence-half 0 is issued right after attention chunks 0-1 +
    # out-proj half 0, so the collective flies while attention chunks 2-3
    # (the expensive ones) compute.
    ar1_in = [nc.dram_tensor(f"ar1_in{h}", [E, SH], BF) for h in range(2)]
    ar1_out = [nc.dram_tensor(f"ar1_out{h}", [E, SH], BF) for h in range(2)]
    at_stack = ExitStack()
    at_pool = at_stack.enter_context(tc.tile_pool(name="attnT", bufs=1))
    attnT = [at_pool.tile([128, S], F32R, tag=f"at{i}", name=f"at{i}") for i in range(OT)]
    with (
        tc.tile_pool(name="pt", bufs=3) as pt_pool,
        tc.tile_pool(name="den", bufs=2) as den_pool,
        tc.tile_pool(name="wo", bufs=1) as wo_pool,
        tc.tile_pool(name="xqs", bufs=4) as xqs_pool,
        tc.tile_pool(name="arin", bufs=4) as arin_pool,
        tc.tile_pool(name="ps_sc", bufs=2, space="PSUM") as ps_sc,
        tc.tile_pool(name="ps_av", bufs=1, space="PSUM") as ps_av,
        tc.tile_pool(name="ps_bc", bufs=1, space="PSUM") as ps_bc,
        tc.tile_pool(name="ps_o", bufs=1, space="PSUM") as ps_o,
    ):
        bo_q = wo_pool.tile([1, E], F32R, tag="bo_q", name="bo_q")
        nc.sync.dma_start(bo_q[:], ins["bo_q"])
        wo_sb = wo_pool.tile([128, OT, E], F32R, tag="wo", name="wo")
        nc.sync.dma_start(
            wo_sb[:], ins["woT_q"].rearrange("(ot p) e -> p ot e", p=128)
        )

        def out_proj_half(half):
            hoff = half * SH
            for et in range(ET):
                for sc in range(SC):
                    coff = hoff + sc * 512
                    psum = ps_o.tile([128, 512], F32, tag="o", name="o")
                    nc.tensor.matmul(
                        psum[:],
                        lhsT=bo_q[:, et * 128 : (et + 1) * 128],
                        rhs=onesrow[:],
                        start=True,
                        stop=False,
                    )
                    for ot in range(OT):
                        nc.tensor.matmul(
                            psum[:],
                            lhsT=wo_sb[:, ot, et * 128 : (et + 1) * 128],
                            rhs=attnT[ot][:, coff : coff + 512],
                            start=False,
                            stop=(ot == OT - 1),
                        )
                    xqs = xqs_pool.tile([128, 512], F32, tag="xqs", name="xqs")
                    nc.sync.dma_start(
                        xqs[:],
                        ins["xq"].bitcast(F32)[
                            et * 128 : (et + 1) * 128, coff : coff + 512
                        ],
                    )
                    arin = arin_pool.tile([128, 512], BF, tag="arin", name="arin")
                    nc.vector.tensor_add(arin[:], psum[:], xqs[:])
                    nc.sync.dma_start(
                        ar1_in[half][
                            et * 128 : (et + 1) * 128, sc * 512 : sc * 512 + 512
                        ],
                        arin[:],
                    )

        def ar1_issue(half):
            if _NO_CC:
                nc.sync.dma_start(ar1_out[half][:], ar1_in[half][:])
                return
            nc.gpsimd.collective_compute(
                "AllReduce",
                OP.add,
                replica_groups=groups,
                ins=[ar1_in[half][:]],
                outs=[ar1_out[half][:]],
            )

        scale = 1.0 / math.sqrt(cfg.Dh)
        for c in range(C4):
            kmax = 4 * c + 4
            for hp in range(HP):
                av = [ps_av.tile([65, 512], F32, tag=f"av{h01}", name=f"av{h01}") for h01 in range(2)]
                for i in range(kmax):
                    psc = ps_sc.tile([128, 1024], F32, tag="sc", name="sc")
                    for h01 in range(2):
                        po = h01 * 64
                        nc.tensor.matmul(
                            psc[:, h01 * 512 : (h01 + 1) * 512],
                            lhsT=qkT[OT + hp][po : po + 64, i * 128 : (i + 1) * 128],
                            rhs=qkT[hp][po : po + 64, c * 512 : (c + 1) * 512],
                            start=True,
                            stop=True,
                            tile_position=(po, 0),
                        )
                    pt = pt_pool.tile([128, 1024], F32R, tag="pt", name="pt")
                    nc.scalar.activation(
                        pt[:], psc[:], AF.Exp,
                        bias=pbias[:, i : i + 1], scale=scale,
                    )
                    r = i - 4 * c
                    if r >= 0:
                        for h01 in range(2):
                            if r > 0:
                                zsl = pt[:, h01 * 512 : h01 * 512 + r * 128]
                                nc.vector.tensor_scalar(
                                    zsl, zsl, 0.0, None, OP.mult
                                )
                            sl = slice(h01 * 512 + r * 128, h01 * 512 + (r + 1) * 128)
                            nc.vector.tensor_mul(pt[:, sl], pt[:, sl], maskd[:])
                    for h01 in range(2):
                        hloc = 2 * hp + h01
                        nc.tensor.matmul(
                            av[h01][:],
                            lhsT=VO[i][:, hloc * 65 : (hloc + 1) * 65],
                            rhs=pt[:, h01 * 512 : (h01 + 1) * 512],
                            start=(i == 0),
                            stop=(i == kmax - 1),
                        )
                dens = [
                    den_pool.tile([1, 512], F32R, tag=f"den{h01}", name=f"den{h01}")
                    for h01 in range(2)
                ]
                with nc.allow_low_precision(reason="f32r rounding for matmul rhs"):
                    for h01 in range(2):
                        nc.vector.reciprocal(dens[h01][:], av[h01][64:65, :])
                # assemble [2, 512] (DMA can place row 1; DVE cannot)
                den2 = den_pool.tile([2, 512], F32R, tag="den2", name="den2")
                for h01 in range(2):
                    nc.sync.dma_start(den2[h01 : h01 + 1, :], dens[h01][:])
                pbc = ps_bc.tile([128, 512], F32, tag="bc", name="bc")
                nc.tensor.matmul(
                    pbc[:], lhsT=onehot2[:], rhs=den2[:], start=True, stop=True
                )
                sbc = den_pool.tile([128, 512], F32, tag="sbc", name="sbc")
                nc.vector.tensor_copy(sbc[:], pbc[:])
                for h01 in range(2):
                    nc.vector.tensor_mul(
                        attnT[hp][h01 * 64 : (h01 + 1) * 64, c * 512 : (c + 1) * 512],
                        av[h01][0:64, :],
                        sbc[h01 * 64 : (h01 + 1) * 64, :],
                    )
            if c == 1:
                out_proj_half(0)
                ar1_issue(0)
        out_proj_half(1)

    at_stack.close()   # attnT no longer needed
    qk_vo_stack.close()  # qkT/VO no longer needed

    # ---------------- LN2 + MLP + AR2 ----------------
    ar2_in = [nc.dram_tensor(f"ar2_in{h}", [E, SH], BF) for h in range(2)]
    ar2_out = [nc.dram_tensor(f"ar2_out{h}", [E, SH], BF) for h in range(2)]
    with (
        tc.tile_pool(name="x2b", bufs=2) as x2b_pool,
        tc.tile_pool(name="x2res", bufs=3) as x2res_pool,
        tc.tile_pool(name="xn2", bufs=1) as xn2_pool,
        tc.tile_pool(name="ht", bufs=1) as ht_pool,
        tc.tile_pool(name="w1", bufs=3) as w1_pool,
        tc.tile_pool(name="w2", bufs=2) as w2_pool,
        tc.tile_pool(name="fin", bufs=2) as fin_pool,
        tc.tile_pool(name="ps_f1", bufs=2, space="PSUM") as ps_f1,
        tc.tile_pool(name="ps_f2", bufs=2, space="PSUM") as ps_f2,
        tc.tile_pool(name="ps_stat", bufs=1, space="PSUM") as ps_stat,
        tc.tile_pool(name="ps_bb", bufs=1, space="PSUM") as ps_bb,
        tc.tile_pool(name="sb_small", bufs=1) as sb_small,
        tc.tile_pool(name="sb_big", bufs=2) as sb_big,
        tc.tile_pool(name="sb_bc", bufs=1) as sb_bc,
    ):
        ln_pools = (ps_stat, ps_bb, sb_small, sb_big, sb_bc)
        bfc2_q = w2_pool.tile([1, E], F32R, tag="bfc2_q", name="bfc2_q")
        nc.sync.dma_start(bfc2_q[:], ins["bfc2_q"])

        def ar2_issue(half):
            if _NO_CC:
                nc.sync.dma_start(ar2_out[half][:], ar2_in[half][:])
            else:
                nc.gpsimd.collective_compute(
                    "AllReduce",
                    OP.add,
                    replica_groups=groups,
                    ins=[ar2_in[half][:]],
                    outs=[ar2_out[half][:]],
                )
            nc.sync.dma_start(outs["outT"][half], ar2_out[half][:])

        def mlp_ln(half):
            xn2_all = xn2_pool.tile([128, ET, SH], F32R, tag="xn2", name="xn2")
            xn2 = [xn2_all[:, et] for et in range(ET)]
            for sc in range(SC):
                coff = sc * 512
                x2b = [
                    x2b_pool.tile([128, 512], BF, tag=f"x2b{et}", name=f"x2b{et}")
                    for et in range(ET)
                ]
                for et in range(ET):
                    nc.sync.dma_start(
                        x2b[et][:],
                        ar1_out[half][
                            et * 128 : (et + 1) * 128, coff : coff + 512
                        ],
                    )
                ln_stats_apply(ln_pools, x2b, 0, eps2, xn2, coff,
                               ones_lhs=ones128b)
            return xn2_all

        def mlp_ffn(half, xn2_all):
            xn2 = [xn2_all[:, et] for et in range(ET)]
            ht_tiles = []
            for ft in range(FT):
                w1t = w1_pool.tile([128, ET, 128], F32R, tag="w1", name="w1")
                nc.sync.dma_start(
                    w1t[:],
                    ins["wfc1T"][:, ft * 128 : (ft + 1) * 128].rearrange(
                        "(et p) f -> p et f", p=128
                    ),
                )
                ht = ht_pool.tile([128, SH], F32R, tag=f"ht{ft}", name=f"ht{ft}")
                for sc in range(SC):
                    psum = ps_f1.tile([128, 512], F32, tag="f1", name="f1")
                    for et in range(ET):
                        nc.tensor.matmul(
                            psum[:],
                            lhsT=w1t[:, et],
                            rhs=xn2[et][:, sc * 512 : (sc + 1) * 512],
                            start=(et == 0),
                            stop=(et == ET - 1),
                        )
                    hsl = ht[:, sc * 512 : (sc + 1) * 512]
                    if cfg.gelu_exact:
                        nc.scalar.activation(
                            hsl, psum[:], AF.Gelu,
                            bias=bfc1[:, ft : ft + 1], scale=1.0,
                        )
                    else:
                        tg = fin_pool.tile([128, 512], F32, tag="tg", name="tg")
                        nc.vector.tensor_scalar(
                            tg[:], psum[:], bfc1[:, ft : ft + 1], None, OP.add
                        )
                        sg = fin_pool.tile([128, 512], F32, tag="sg", name="sg")
                        nc.scalar.activation(sg[:], tg[:], AF.Sigmoid, scale=1.702)
                        nc.vector.tensor_mul(hsl, tg[:], sg[:])
                ht_tiles.append(ht)
            for et in range(ET):
                w2t = w2_pool.tile([128, FT, 128], F32R, tag="w2", name="w2")
                nc.sync.dma_start(
                    w2t[:],
                    ins["wfc2T"][:, et * 128 : (et + 1) * 128].rearrange(
                        "(ft p) e -> p ft e", p=128
                    ),
                )
                fin = fin_pool.tile([128, SH], BF, tag="fin", name="fin")
                for sc in range(SC):
                    psum = ps_f2.tile([128, 512], F32, tag="f2", name="f2")
                    nc.tensor.matmul(
                        psum[:],
                        lhsT=bfc2_q[:, et * 128 : (et + 1) * 128],
                        rhs=onesrow[:],
                        start=True,
                        stop=False,
                    )
                    for ft in range(FT):
                        nc.tensor.matmul(
                            psum[:],
                            lhsT=w2t[:, ft],
                            rhs=ht_tiles[ft][:, sc * 512 : (sc + 1) * 512],
                            start=False,
                            stop=(ft == FT - 1),
                        )
                    x2res = x2res_pool.tile([128, 512], BF, tag="x2res", name="x2res")
                    nc.sync.dma_start(
                        x2res[:],
                        ar1_out[half][
                            et * 128 : (et + 1) * 128,
                            sc * 512 : (sc + 1) * 512,
                        ],
                    )
                    nc.vector.tensor_add(
                        fin[:, sc * 512 : (sc + 1) * 512], psum[:], x2res[:]
                    )
                nc.sync.dma_start(
                    ar2_in[half][et * 128 : (et + 1) * 128, :], fin[:]
                )

        xn2_0 = mlp_ln(0)
        ar1_issue(1)   # late issue: keeps h0 LN reads off the AR1h1 wait
        mlp_ffn(0, xn2_0)
        xn2_1 = mlp_ln(1)
        ar2_issue(0)   # late issue: keeps h1 LN reads off the AR2h0 wait
        mlp_ffn(1, xn2_1)
        ar2_issue(1)


# ---------------------------------------------------------------------------
# host side
# ---------------------------------------------------------------------------

def prep_inputs(cfg: Cfg, x, td, ln1_g, ln1_b, ln2_g, ln2_b, w_qkv, b_qkv,
                w_o, b_o, w_fc1, b_fc1, w_fc2, b_fc2):
    """Build the per-core input maps (numpy, fp32)."""
    E, H, FL, OL, HL = cfg.E, cfg.H, cfg.FL, cfg.OL, cfg.HL
    f4 = np.float32
    asc = np.ascontiguousarray

    wq, wk, wv = w_qkv[0:E], w_qkv[E : 2 * E], w_qkv[2 * E : 3 * E]
    bq, bk, bvv = b_qkv[0:E], b_qkv[E : 2 * E], b_qkv[2 * E : 3 * E]

    shared = {}
    shared["onesrow"] = np.ones((1, 512), f4)
    oh = np.zeros((2, 128), f4)
    oh[0, 0:64] = 1.0
    oh[1, 64:128] = 1.0
    shared["onehot2"] = oh
    shared["ones128"] = np.ones((128, 8), f4)
    import ml_dtypes

    shared["ones128b"] = np.ones((128, 8), ml_dtypes.bfloat16)
    k_idx = np.arange(128)
    shared["maskd"] = asc((k_idx[:, None] <= k_idx[None, :]).astype(f4))
    pb = np.zeros((128, cfg.ST), f4)
    for i in range(cfg.ST):
        kabs = i * 128 + k_idx
        pb[(kabs % td) == (td - 1), i] = NEG
    shared["pbias"] = pb
    shared["bo_q"] = asc(0.25 * b_o[None, :].astype(f4))
    shared["bfc2_q"] = asc(0.5 * b_fc2[None, :].astype(f4))

    per_tp = []
    for tp in range(2):
        o_sl = slice(tp * OL, (tp + 1) * OL)
        f_sl = slice(tp * FL, (tp + 1) * FL)
        d = {}
        wqk = np.concatenate([wq[o_sl], wk[o_sl]], axis=0)  # [2*OL, E]
        d["wqkT"] = asc((wqk * ln1_g[None, :]).T.astype(f4))  # [E, 2*OL]
        bqk_full = (
            np.concatenate([bq[o_sl], bk[o_sl]]) + wqk @ ln1_b
        ).astype(f4)  # [2*OL]
        d["bqk"] = asc(bqk_full.reshape(2 * cfg.OT, 128).T)  # [128, 2*OT]
        d["wvT"] = asc((wv[o_sl] * ln1_g[None, :]).T.astype(f4))  # [E, OL]
        d["bv"] = asc((bvv[o_sl] + wv[o_sl] @ ln1_b)[None, :].astype(f4))  # [1, OL]
        d["woT_q"] = asc(0.5 * w_o[:, o_sl].T.astype(f4))  # [OL, E]
        d["wfc1T"] = asc((w_fc1[f_sl] * ln2_g[None, :]).T.astype(f4))  # [E, FL]
        d["bfc1"] = asc(
            (b_fc1[f_sl] + w_fc1[f_sl] @ ln2_b).astype(f4).reshape(cfg.FT, 128).T
        )  # [128, FT]
        d["wfc2T"] = asc(w_fc2[:, f_sl].T.astype(f4))  # [FL, E]
        per_tp.append(d)

    in_maps = []
    for c in range(2 * cfg.n_pairs):
        p, tp = c // 2, c % 2
        m = dict(shared)
        m.update(per_tp[tp])
        m["xq"] = asc(0.25 * x[p].T.astype(f4))  # [E, S]
        in_maps.append(m)
    return in_maps


_F32R_INPUTS = {
    "xq", "wqkT", "wvT", "bv", "woT_q", "bo_q", "wfc1T", "wfc2T", "bfc2_q",
    "onesrow", "onehot2", "ones128", "maskd",
}
_BF16_INPUTS = {"ones128b"}
_F8_INPUTS = set()


def build_nc(cfg: Cfg, sample_map):
    nc = bacc.Bacc(
        "TRN2", target_bir_lowering=False, debug=False,
        num_devices=2 * cfg.n_pairs,
    )
    ins = {}
    for name, arr in sample_map.items():
        dt_ = (
            F8 if name in _F8_INPUTS
            else BF if name in _BF16_INPUTS
            else F32R if name in _F32R_INPUTS
            else F32
        )
        ins[name] = nc.dram_tensor(
            name, list(arr.shape), dt_, kind="ExternalInput"
        ).ap()
    outs = {
        "outT": nc.dram_tensor(
            "outT", [2, cfg.E, cfg.SH], BF, kind="ExternalOutput"
        ).ap()
    }
    with tile.TileContext(nc) as tc:
        block_kernel(tc, cfg, ins, outs)
    nc.compile()
    return nc


_CACHE = {}


def _get_nc(cfg: Cfg, sample_map):
    if cfg not in _CACHE:
        _CACHE[cfg] = build_nc(cfg, sample_map)
    return _CACHE[cfg]


def assemble_output(cfg: Cfg, results):
    """results: list of per-core output dicts -> full [B, S, E]."""
    out = np.empty((cfg.B, cfg.S, cfg.E), np.float32)
    for p in range(cfg.n_pairs):
        oT = np.asarray(results[2 * p]["outT"], np.float32)  # [2, E, SH]
        out[p] = np.concatenate([oT[0], oT[1]], axis=1).T
    return out


class Runner:
    """Cached PJRT runner (mirrors bass2jax.run_bass_via_pjrt multi-core
    path, but keeps the jitted executable and device-resident inputs so
    repeated calls don't re-trace / re-transfer)."""

    def __init__(self, nc, n_cores):
        import jax
        from jax.sharding import Mesh, PartitionSpec
        from jax.experimental.shard_map import shard_map
        from concourse import bass2jax, mybir as mb

        bass2jax.install_neuronx_cc_hook()
        self.nc = nc
        self.n_cores = n_cores
        partition_name = (
            nc.partition_id_tensor.name if nc.partition_id_tensor else None
        )
        in_names, out_names, out_avals, zero_outs = [], [], [], []
        for alloc in nc.m.functions[0].allocations:
            if not isinstance(alloc, mb.MemoryLocationSet):
                continue
            name = alloc.memorylocations[0].name
            if alloc.kind == "ExternalInput":
                if name != partition_name:
                    in_names.append(name)
            elif alloc.kind == "ExternalOutput":
                shape = tuple(alloc.tensor_shape)
                dtype = mb.dt.np(alloc.dtype)
                out_names.append(name)
                out_avals.append(jax.core.ShapedArray(shape, dtype))
                zero_outs.append(np.zeros(shape, dtype))
        self.in_names = list(in_names)
        self.out_names = out_names
        self.out_avals = out_avals
        self.zero_outs = zero_outs
        n_params = len(self.in_names)
        all_in = list(self.in_names) + list(out_names)
        if partition_name is not None:
            all_in.append(partition_name)
        donate = tuple(range(n_params, n_params + len(out_names)))

        def _body(*args):
            operands = list(args)
            if partition_name is not None:
                operands.append(bass2jax.partition_id_tensor())
            outs = bass2jax._bass_exec_p.bind(
                *operands,
                out_avals=tuple(out_avals),
                in_names=tuple(all_in),
                out_names=tuple(out_names),
                lowering_input_output_aliases=(),
                sim_require_finite=True,
                sim_require_nnan=True,
                nc=nc,
            )
            return tuple(outs)

        devices = jax.devices()[:n_cores]
        self.mesh = Mesh(np.asarray(devices), ("core",))
        in_specs = (PartitionSpec("core"),) * (n_params + len(out_names))
        out_specs = (PartitionSpec("core"),) * len(out_names)
        self.sharded = jax.jit(
            shard_map(
                _body, mesh=self.mesh, in_specs=in_specs, out_specs=out_specs,
                check_rep=False,
            ),
            donate_argnums=donate,
            keep_unused=True,
        )
        self._jax = jax

    def concat_inputs(self, in_maps):
        return [
            np.concatenate(
                [np.asarray(in_maps[c][n]) for c in range(self.n_cores)], axis=0
            )
            for n in self.in_names
        ]

    def fresh_zeros(self):
        return [
            np.zeros((self.n_cores * z.shape[0], *z.shape[1:]), z.dtype)
            for z in self.zero_outs
        ]

    def run(self, concat_in, zeros=None):
        if zeros is None:
            zeros = self.fresh_zeros()
        out_arrs = self.sharded(*concat_in, *zeros)
        return [
            {
                name: np.asarray(out_arrs[i]).reshape(
                    self.n_cores, *self.out_avals[i].shape
                )[c]
                for i, name in enumerate(self.out_names)
            }
            for c in range(self.n_cores)
        ]


_RUNNER = {}


def get_runner(cfg: Cfg, sample_map):
    if cfg not in _RUNNER:
        _RUNNER[cfg] = Runner(_get_nc(cfg, sample_map), 2 * cfg.n_pairs)
    return _RUNNER[cfg]


def make_in_maps(cfg: Cfg, inputs):
    x = np.asarray(inputs["x"], np.float32)
    td = int(np.asarray(inputs["transition_dim"]))
    return prep_inputs(
        cfg, x, td,
        np.asarray(inputs["ln1_g"], np.float32),
        np.asarray(inputs["ln1_b"], np.float32),
        np.asarray(inputs["ln2_g"], np.float32),
        np.asarray(inputs["ln2_b"], np.float32),
        np.asarray(inputs["w_qkv"], np.float32),
        np.asarray(inputs["b_qkv"], np.float32),
        np.asarray(inputs["w_o"], np.float32),
        np.asarray(inputs["b_o"], np.float32),
        np.asarray(inputs["w_fc1"], np.float32),
        np.asarray(inputs["b_fc1"], np.float32),
        np.asarray(inputs["w_fc2"], np.float32),
        np.asarray(inputs["b_fc2"], np.float32),
    )


def kernel(**inputs) -> np.ndarray:
    cfg = Cfg()
    in_maps = make_in_maps(cfg, inputs)
    runner = get_runner(cfg, in_maps[0])
    results = runner.run(runner.concat_inputs(in_maps))
    return assemble_output(cfg, results)

